# revision 2
# baseline (speedup 1.0000x reference)
"""Trainium2 Bass kernel for a ViT-Base transformer encoder block.

Input x: [64, 197, 768] fp32 + weights. Data-parallel over batch across 8
NeuronCores (8 batches/core = 1576 tokens/core). Per core:

  pass 1: QKV projections (fp32r matmuls, d-major q/k, token-major v),
          attention per batch-pair (2 batches packed into the N=394 moving dim
          so fp32r runs at full rate), softmax without max-subtraction
          (scores are O(1) here), O-projection, LayerNorm1 + residual -> x1
          (spilled to DRAM scratch).
  pass 2: MLP (W1 resident, W2 streamed), exact GELU fused into the PSUM
          eviction, d-major MLP2 + PE transpose back to token-major,
          LayerNorm2 + residual -> out.
"""
import os
import sys

sys.path.insert(0, "/opt/trn_rl_repo")

import numpy as np
from contextlib import ExitStack

import concourse.bass as bass
import concourse.tile as tile
from concourse import bacc, mybir
from concourse.bass_utils import run_bass_kernel_spmd
from concourse.masks import make_identity
from concourse.tile_rust import add_dep_helper

DIM, NH, HD, HID = 768, 12, 64, 3072
S = 197
B = 64
N_CORES = 8
BPC = B // N_CORES            # 8 batches per core
T = BPC * S                   # 1576 tokens per core
NPAIR = BPC // 2              # 4 batch pairs per core
PT = 2 * S                    # 394 tokens per pair
EPS = 1e-6
DC = DIM // 128               # 6 d-chunks
HC = HID // 128               # 24 hidden chunks

F32 = mybir.dt.float32
F32R = mybir.dt.float32r
AF = mybir.ActivationFunctionType
OP = mybir.AluOpType

# token tiles within a pair: (offset, size); tile i = 2*b + kt for batch b
Q_TILES = [(0, 128), (128, 69), (197, 128), (325, 69)]

DEBUG = bool(int(os.environ.get("BASSK_DEBUG", "0")))

_cached = None


def _build():
    nc = bacc.Bacc("TRN2", target_bir_lowering=False, debug=False)

    x_d = nc.dram_tensor("x", [T, DIM], F32, kind="ExternalInput").ap()
    w_d = {}
    for name, shape in [("Wq", [DIM, DIM]), ("Wk", [DIM, DIM]),
                        ("Wv", [DIM, DIM]), ("Wo", [DIM, DIM]),
                        ("W1", [DIM, HID]), ("W2", [HID, DIM]),
                        ("bq", [DIM]), ("bk", [DIM]), ("bv", [DIM]),
                        ("bo", [DIM]), ("b1", [HID]), ("b2", [DIM]),
                        ("g1", [DIM]), ("be1", [DIM]), ("g2", [DIM]),
                        ("be2", [DIM])]:
        w_d[name] = nc.dram_tensor(name, shape, F32, kind="ExternalInput").ap()
    out_d = nc.dram_tensor("out", [T, DIM], F32, kind="ExternalOutput").ap()
    x1_d = nc.dram_tensor("x1s", [T, DIM], F32).ap()  # internal scratch

    dbg = {}
    if DEBUG:
        for name, shape in [("dq", [DIM, T]), ("dk", [DIM, T]),
                            ("dv", [T, DIM]), ("dctx", [DIM, T]),
                            ("dx1", [T, DIM]), ("dh", [HID, PT])]:
            dbg[name] = nc.dram_tensor(name, shape, F32, kind="ExternalOutput").ap()

    x1_store_insts = {}

    with tile.TileContext(nc) as tc, ExitStack() as octx:
        persist = octx.enter_context(tc.tile_pool(name="persist", bufs=1))

        # ---- constants ----
        ident = persist.tile([128, 128], F32)
        make_identity(nc, ident[:])
        ones_f = persist.tile([128, 64], F32)
        nc.vector.memset(ones_f[:], 1.0)
        ones_r = persist.tile([128, 64], F32R)
        nc.vector.tensor_copy(ones_r[:], ones_f[:])
        eps_sb = persist.tile([128, 1], F32)
        nc.vector.memset(eps_sb[:], EPS)

        # per-partition bias layouts [128, nchunk]
        bq_sb = persist.tile([128, DC], F32)
        nc.sync.dma_start(bq_sb[:], w_d["bq"].rearrange("(c p) -> p c", p=128))
        bk_sb = persist.tile([128, DC], F32)
        nc.sync.dma_start(bk_sb[:], w_d["bk"].rearrange("(c p) -> p c", p=128))
        b1_sb = persist.tile([128, HC], F32)
        nc.sync.dma_start(b1_sb[:], w_d["b1"].rearrange("(c p) -> p c", p=128))

        # broadcast-[128, 768] biases
        bcast = {}
        for name in ["bv", "bo", "b2", "g1", "be1", "g2", "be2"]:
            tb = persist.tile([128, DIM], F32, name=f"bc_{name}", tag=f"bc_{name}")
            nc.sync.dma_start(tb[:], w_d[name].unsqueeze(0).to_broadcast([128, DIM]))
            bcast[name] = tb

        # =========================== PASS 1 ===========================
        with ExitStack() as ctx:
            wpool = ctx.enter_context(tc.tile_pool(name="w1p", bufs=1))
            Wt = {}
            for name in ["Wq", "Wk", "Wv", "Wo"]:
                wt = wpool.tile([128, DC, DIM], F32R, name=f"wt_{name}", tag=f"wt_{name}")
                for c in range(DC):
                    nc.sync.dma_start(wt[:, c, :],
                                      w_d[name][c * 128:(c + 1) * 128, :].bitcast(F32R))
                Wt[name] = wt

            big = ctx.enter_context(tc.tile_pool(name="p1big", bufs=1))
            exp_pool = ctx.enter_context(tc.tile_pool(name="exp", bufs=4))
            rt_pool = ctx.enter_context(tc.tile_pool(name="rt", bufs=2))
            ao_pool = ctx.enter_context(tc.tile_pool(name="ao", bufs=2))
            ln_pool = ctx.enter_context(tc.tile_pool(name="ln", bufs=3))
            x1_pool = ctx.enter_context(tc.tile_pool(name="x1", bufs=2))

            ps_mm = ctx.enter_context(tc.tile_pool(name="psmm", bufs=2, space="PSUM"))
            ps_sc = ctx.enter_context(tc.tile_pool(name="pssc", bufs=2, space="PSUM"))
            ps_cx = ctx.enter_context(tc.tile_pool(name="pscx", bufs=2, space="PSUM"))
            ps_bc = ctx.enter_context(tc.tile_pool(name="psbc", bufs=2, space="PSUM"))

            for p in range(NPAIR):
                g0 = p * PT

                # ---- load x (token-major) ----
                x_sb = big.tile([128, 4, DIM], F32, tag="x")
                for i, (off, sz) in enumerate(Q_TILES):
                    nc.sync.dma_start(x_sb[0:sz, i, :], x_d[g0 + off:g0 + off + sz, :])

                # ---- transpose x -> xT (d-major) ----
                xT = big.tile([128, DC, PT], F32R, tag="xT")
                for i, (off, sz) in enumerate(Q_TILES):
                    for c in range(DC):
                        pt = ps_mm.tile([128, 394], F32, tag="mm")
                        nc.tensor.transpose(pt[:, 0:sz],
                                            x_sb[0:sz, i, c * 128:(c + 1) * 128],
                                            ident[0:sz, 0:sz])
                        nc.vector.tensor_copy(xT[:, c, off:off + sz], pt[:, 0:sz])

                # ---- Q/K projections (d-major) ----
                qT = big.tile([128, DC, PT], F32R, tag="qT")
                kT = big.tile([128, DC, PT], F32R, tag="kT")
                for c in range(DC):
                    for wname, bsb, dst in (("Wq", bq_sb, qT), ("Wk", bk_sb, kT)):
                        pm = ps_mm.tile([128, 394], F32, tag="mm")
                        for kc in range(DC):
                            nc.tensor.matmul(pm[:, :],
                                             Wt[wname][:, kc, c * 128:(c + 1) * 128],
                                             xT[:, kc, :],
                                             start=(kc == 0), stop=(kc == DC - 1))
                        nc.scalar.activation(dst[:, c, :], pm[:, :], AF.Identity,
                                             bias=bsb[:, c:c + 1], scale=1.0)

                # ---- V projection (token-major, 65-stride head layout) ----
                v_sb = big.tile([128, 4, NH, HD + 1], F32R, tag="v")
                for i, (off, sz) in enumerate(Q_TILES):
                    for s in range(2):
                        pm = ps_mm.tile([128, 394], F32, tag="mm")
                        for kc in range(DC):
                            nc.tensor.matmul(pm[0:sz, 0:384],
                                             xT[:, kc, off:off + sz],
                                             Wt["Wv"][:, kc, s * 384:(s + 1) * 384],
                                             start=(kc == 0), stop=(kc == DC - 1))
                        nc.vector.tensor_add(
                            v_sb[0:sz, i, 6 * s:6 * s + 6, 0:HD],
                            pm[0:sz, 0:384].rearrange("p (a b) -> p a b", a=6),
                            bcast["bv"][0:sz, s * 384:(s + 1) * 384]
                                .rearrange("p (a b) -> p a b", a=6))

                if DEBUG and p == 0:
                    for c in range(DC):
                        nc.sync.dma_start(dbg["dq"][c * 128:(c + 1) * 128, 0:PT],
                                          qT[:, c, :].bitcast(F32))
                        nc.sync.dma_start(dbg["dk"][c * 128:(c + 1) * 128, 0:PT],
                                          kT[:, c, :].bitcast(F32))
                    for i, (off, sz) in enumerate(Q_TILES):
                        nc.sync.dma_start(
                            dbg["dv"][off:off + sz, :],
                            v_sb[0:sz, i, :, 0:HD].bitcast(F32))

                # ---- attention (per head; 2 batches packed in N) ----
                ctxT = big.tile([128, DC, PT], F32R, tag="ctxT")
                for h in range(NH):
                    hc, hp = h // 2, (h % 2) * 64
                    exps = []  # tile per (b, kt)
                    for b in range(2):
                        for kt in range(2):
                            koff, ksz = Q_TILES[2 * b + kt]
                            psc = ps_sc.tile([128, 394], F32, tag="sc")
                            nc.tensor.matmul(psc[0:ksz, :],
                                             kT[hp:hp + 64, hc, koff:koff + ksz],
                                             qT[hp:hp + 64, hc, :],
                                             start=True, stop=True)
                            et = exp_pool.tile([128, 394], F32R, tag="exp")
                            nc.scalar.activation(et[0:ksz, :], psc[0:ksz, :],
                                                 AF.Exp, bias=0.0, scale=0.125)
                            exps.append(et)

                    # unnormalized ctx^T at base 0, one psum per batch
                    pcs = []
                    for b in range(2):
                        pc = ps_cx.tile([128, 394], F32, tag="cx")
                        for kt in range(2):
                            _, ksz = Q_TILES[2 * b + kt]
                            nc.tensor.matmul(
                                pc[0:HD, :],
                                v_sb[0:ksz, 2 * b + kt, h, 0:HD],
                                exps[2 * b + kt][0:ksz, :],
                                start=(kt == 0), stop=(kt == 1))
                        pcs.append(pc)

                    # softmax denominators: rowsumT [1, 394] per batch -> recip
                    rt = rt_pool.tile([128, 394], F32R, tag="rt")
                    with nc.allow_low_precision(reason="softmax recip feeds fp32r matmul"):
                        for b in range(2):
                            pr = ps_mm.tile([128, 394], F32, tag="mm")
                            for kt in range(2):
                                _, ksz = Q_TILES[2 * b + kt]
                                nc.tensor.matmul(pr[0:1, :],
                                                 ones_r[0:ksz, 0:1],
                                                 exps[2 * b + kt][0:ksz, :],
                                                 start=(kt == 0), stop=(kt == 1))
                            nc.vector.reciprocal(
                                rt[0:1, b * S:(b + 1) * S],
                                pr[0:1, b * S:(b + 1) * S])

                    # broadcast recip over 64 partitions (base 0)
                    pb = ps_bc.tile([128, 394], F32, tag="bc")
                    nc.tensor.matmul(pb[0:HD, :],
                                     ones_r[0:1, 0:HD],
                                     rt[0:1, :],
                                     start=True, stop=True)
                    bc_sb = rt_pool.tile([128, 394], F32, tag="bcsb")
                    nc.scalar.activation(bc_sb[0:HD, :], pb[0:HD, :],
                                         AF.Copy, bias=0.0, scale=1.0)
                    if hp == 0:
                        for b in range(2):
                            cs = slice(b * S, (b + 1) * S)
                            nc.vector.tensor_tensor(
                                ctxT[0:HD, hc, cs],
                                pcs[b][0:HD, cs],
                                bc_sb[0:HD, cs], OP.mult)
                    else:
                        # normalize at base 0, then DMA-shift to partitions 64:128
                        nrm = rt_pool.tile([128, 394], F32R, tag="nrm")
                        for b in range(2):
                            cs = slice(b * S, (b + 1) * S)
                            nc.vector.tensor_tensor(
                                nrm[0:HD, cs],
                                pcs[b][0:HD, cs],
                                bc_sb[0:HD, cs], OP.mult)
                        nc.sync.dma_start(ctxT[64:128, hc, :], nrm[0:HD, :])

                if DEBUG and p == 0:
                    for c in range(DC):
                        nc.sync.dma_start(dbg["dctx"][c * 128:(c + 1) * 128, 0:PT],
                                          ctxT[:, c, :].bitcast(F32))

                # ---- O-projection + LN1 + residual ----
                for i, (off, sz) in enumerate(Q_TILES):
                    ao = ao_pool.tile([128, DIM], F32, tag="ao")
                    for s in range(2):
                        pm = ps_mm.tile([128, 394], F32, tag="mm")
                        for c in range(DC):
                            nc.tensor.matmul(pm[0:sz, 0:384],
                                             ctxT[:, c, off:off + sz],
                                             Wt["Wo"][:, c, s * 384:(s + 1) * 384],
                                             start=(c == 0), stop=(c == DC - 1))
                        nc.vector.tensor_add(ao[0:sz, s * 384:(s + 1) * 384],
                                             pm[0:sz, 0:384],
                                             bcast["bo"][0:sz, s * 384:(s + 1) * 384])
                    # LayerNorm 1
                    st = ln_pool.tile([128, 3, nc.vector.BN_STATS_DIM], F32, tag="st")
                    for g in range(3):
                        nc.vector.bn_stats(st[0:sz, g, :], ao[0:sz, g * 256:(g + 1) * 256])
                    mv = ln_pool.tile([128, nc.vector.BN_AGGR_DIM], F32, tag="mv")
                    nc.vector.bn_aggr(mv[0:sz, :], st[0:sz, :, :])
                    rstd = ln_pool.tile([128, 1], F32, tag="rstd")
                    nc.scalar.activation(rstd[0:sz, :], mv[0:sz, 1:2], AF.Sqrt,
                                         bias=eps_sb[0:sz, :], scale=1.0)
                    nc.vector.reciprocal(rstd[0:sz, :], rstd[0:sz, :])
                    nmr = ln_pool.tile([128, 1], F32, tag="nmr")
                    nc.vector.tensor_scalar(nmr[0:sz, :], mv[0:sz, 0:1],
                                            rstd[0:sz, :], -1.0, OP.mult, OP.mult)
                    tln = ao_pool.tile([128, DIM], F32, tag="tln")
                    nc.scalar.activation(tln[0:sz, :], ao[0:sz, :], AF.Identity,
                                         bias=nmr[0:sz, :], scale=rstd[0:sz, :])
                    x1t = x1_pool.tile([128, DIM], F32, tag="x1")
                    nc.vector.tensor_tensor(x1t[0:sz, :], tln[0:sz, :],
                                            bcast["g1"][0:sz, :], OP.mult)
                    nc.vector.tensor_add(x1t[0:sz, :], x1t[0:sz, :], x_sb[0:sz, i, :])
                    nc.vector.tensor_add(x1t[0:sz, :], x1t[0:sz, :],
                                         bcast["be1"][0:sz, :])
                    x1_store_insts[(p, i)] = nc.sync.dma_start(
                        x1_d[g0 + off:g0 + off + sz, :], x1t[0:sz, :])
                    if DEBUG and p == 0:
                        nc.sync.dma_start(dbg["dx1"][off:off + sz, :], x1t[0:sz, :])

        # =========================== PASS 2 ===========================
        with ExitStack() as ctx:
            wpool = ctx.enter_context(tc.tile_pool(name="w2p", bufs=1))
            W1t = wpool.tile([128, DC, HID], F32R)
            for c in range(DC):
                nc.sync.dma_start(W1t[:, c, :],
                                  w_d["W1"][c * 128:(c + 1) * 128, :].bitcast(F32R))

            big = ctx.enter_context(tc.tile_pool(name="p2big", bufs=1))
            w2_pool = ctx.enter_context(tc.tile_pool(name="w2s", bufs=4))
            mo_pool = ctx.enter_context(tc.tile_pool(name="mo", bufs=2))
            ln_pool = ctx.enter_context(tc.tile_pool(name="ln2", bufs=3))
            out_pool = ctx.enter_context(tc.tile_pool(name="outp", bufs=2))

            ps_wk = ctx.enter_context(tc.tile_pool(name="pswk", bufs=2, space="PSUM"))
            ps_ac = ctx.enter_context(tc.tile_pool(name="psac", bufs=6, space="PSUM"))

            for p in range(NPAIR):
                g0 = p * PT

                x1_sb = big.tile([128, 4, DIM], F32, tag="x1r")
                for i, (off, sz) in enumerate(Q_TILES):
                    ld = nc.sync.dma_start(x1_sb[0:sz, i, :],
                                           x1_d[g0 + off:g0 + off + sz, :])
                    add_dep_helper(ld.ins, x1_store_insts[(p, i)].ins,
                                   sync=True, reason="x1 dram roundtrip RAW")

                x1T = big.tile([128, DC, PT], F32R, tag="x1T")
                for i, (off, sz) in enumerate(Q_TILES):
                    for c in range(DC):
                        pt = ps_wk.tile([128, 394], F32, tag="wk")
                        nc.tensor.transpose(pt[:, 0:sz],
                                            x1_sb[0:sz, i, c * 128:(c + 1) * 128],
                                            ident[0:sz, 0:sz])
                        nc.vector.tensor_copy(x1T[:, c, off:off + sz], pt[:, 0:sz])

                # ---- MLP1 (d-major h) + fused bias+GELU, and MLP2 accumulation ----
                hT = big.tile([128, HC, PT], F32R, tag="hT")
                pacs = [ps_ac.tile([128, 394], F32, tag="ac", name=f"pac{c}") for c in range(DC)]
                for hcx in range(HC):
                    pm = ps_wk.tile([128, 394], F32, tag="wk")
                    for kc in range(DC):
                        nc.tensor.matmul(pm[:, :],
                                         W1t[:, kc, hcx * 128:(hcx + 1) * 128],
                                         x1T[:, kc, :],
                                         start=(kc == 0), stop=(kc == DC - 1))
                    nc.scalar.activation(hT[:, hcx, :], pm[:, :], AF.Gelu,
                                         bias=b1_sb[:, hcx:hcx + 1], scale=1.0)
                    # stream W2 chunk and accumulate all 6 output chunks
                    w2c = w2_pool.tile([128, DIM], F32R, tag="w2c")
                    nc.sync.dma_start(
                        w2c[:], w_d["W2"][hcx * 128:(hcx + 1) * 128, :].bitcast(F32R))
                    for c in range(DC):
                        nc.tensor.matmul(pacs[c][:, :],
                                         w2c[:, c * 128:(c + 1) * 128],
                                         hT[:, hcx, :],
                                         start=(hcx == 0), stop=(hcx == HC - 1))

                if DEBUG and p == 0:
                    for hcx in range(HC):
                        nc.sync.dma_start(dbg["dh"][hcx * 128:(hcx + 1) * 128, :],
                                          hT[:, hcx, :].bitcast(F32))

                # ---- evict mlp_outT, transpose back to token-major ----
                moT = big.tile([128, DC, PT], F32, tag="moT")
                for c in range(DC):
                    nc.scalar.activation(moT[:, c, :], pacs[c][:, :], AF.Copy,
                                         bias=0.0, scale=1.0)

                for i, (off, sz) in enumerate(Q_TILES):
                    mo = mo_pool.tile([128, DIM], F32, tag="mo")
                    for c in range(DC):
                        pt = ps_wk.tile([128, 394], F32, tag="wk")
                        nc.tensor.transpose(pt[0:sz, 0:128],
                                            moT[:, c, off:off + sz], ident[:, :])
                        nc.vector.tensor_copy(mo[0:sz, c * 128:(c + 1) * 128],
                                              pt[0:sz, 0:128])
                    nc.vector.tensor_add(mo[0:sz, :], mo[0:sz, :], bcast["b2"][0:sz, :])
                    # LayerNorm 2 + residual
                    st = ln_pool.tile([128, 3, nc.vector.BN_STATS_DIM], F32, tag="st")
                    for g in range(3):
                        nc.vector.bn_stats(st[0:sz, g, :], mo[0:sz, g * 256:(g + 1) * 256])
                    mv = ln_pool.tile([128, nc.vector.BN_AGGR_DIM], F32, tag="mv")
                    nc.vector.bn_aggr(mv[0:sz, :], st[0:sz, :, :])
                    rstd = ln_pool.tile([128, 1], F32, tag="rstd")
                    nc.scalar.activation(rstd[0:sz, :], mv[0:sz, 1:2], AF.Sqrt,
                                         bias=eps_sb[0:sz, :], scale=1.0)
                    nc.vector.reciprocal(rstd[0:sz, :], rstd[0:sz, :])
                    nmr = ln_pool.tile([128, 1], F32, tag="nmr")
                    nc.vector.tensor_scalar(nmr[0:sz, :], mv[0:sz, 0:1],
                                            rstd[0:sz, :], -1.0, OP.mult, OP.mult)
                    tln = mo_pool.tile([128, DIM], F32, tag="tln")
                    nc.scalar.activation(tln[0:sz, :], mo[0:sz, :], AF.Identity,
                                         bias=nmr[0:sz, :], scale=rstd[0:sz, :])
                    ot = out_pool.tile([128, DIM], F32, tag="ot")
                    nc.vector.tensor_tensor(ot[0:sz, :], tln[0:sz, :],
                                            bcast["g2"][0:sz, :], OP.mult)
                    nc.vector.tensor_add(ot[0:sz, :], ot[0:sz, :], x1_sb[0:sz, i, :])
                    nc.vector.tensor_add(ot[0:sz, :], ot[0:sz, :],
                                         bcast["be2"][0:sz, :])
                    nc.sync.dma_start(out_d[g0 + off:g0 + off + sz, :], ot[0:sz, :])

    nc.compile()
    return nc


def kernel(x, Wq, bq, Wk, bk, Wv, bv, Wo, bo, W1, b1, W2, b2, g1, be1, g2, be2):
    global _cached
    if _cached is None:
        _cached = _build()
    nc = _cached

    weights = dict(Wq=Wq, bq=bq, Wk=Wk, bk=bk, Wv=Wv, bv=bv, Wo=Wo, bo=bo,
                   W1=W1, b1=b1, W2=W2, b2=b2, g1=g1, be1=be1, g2=g2, be2=be2)
    weights = {k: np.ascontiguousarray(v, dtype=np.float32) for k, v in weights.items()}
    x = np.asarray(x, dtype=np.float32)

    in_maps = []
    for c in range(N_CORES):
        xc = np.ascontiguousarray(
            x[c * BPC:(c + 1) * BPC].reshape(T, DIM))
        in_maps.append({"x": xc, **weights})

    res = run_bass_kernel_spmd(nc, in_maps, core_ids=list(range(N_CORES)),
                               trace=bool(int(os.environ.get("BASSK_TRACE", "0"))))
    kernel._last_res = res
    out = np.concatenate(
        [res.results[c]["out"].reshape(BPC, S, DIM) for c in range(N_CORES)], axis=0)
    return out.astype(np.float32)



# revision 17
# speedup vs baseline: 1.4806x; 1.4806x over previous
"""Trainium2 Bass kernel for a ViT-Base transformer encoder block.

Input x: [64, 197, 768] fp32 + weights. Data-parallel over batch across 8
NeuronCores (8 batches/core = 1576 tokens/core). Single fused pass per
batch-pair (394 tokens), 4 pairs per core:

  x -> (bf16) xT -> Q/K/V projections (bf16 matmuls) -> attention with
  per-batch moving dim 197 (bf16), row-sums accumulated into a shared PSUM
  via ones-matmuls, one reciprocal for all heads, PE-broadcast of the
  per-(head,token) reciprocals via a 0/1 selection matrix -> O-projection ->
  LN1+residual -> x1 (bf16, kept in SBUF) -> MLP (bf16, W1/W2 resident,
  interleaved MLP1/MLP2 with 6-bank PSUM accumulation) -> transpose back ->
  LN2+residual -> out.

All weights are converted to bf16 on-chip once (staged fp32 DMA + cast).
"""
import os
import sys

sys.path.insert(0, "/opt/trn_rl_repo")

import numpy as np
from contextlib import ExitStack

import concourse.bass as bass
import concourse.tile as tile
from concourse import bacc, mybir
from concourse.bass_utils import run_bass_kernel_spmd
from concourse.masks import make_identity

DIM, NH, HD, HID = 768, 12, 64, 3072
S = 197
B = 64
N_CORES = 8
BPC = B // N_CORES            # 8 batches per core
T = BPC * S                   # 1576 tokens per core
NPAIR = BPC // 2              # 4 batch pairs per core
PT = 2 * S                    # 394 tokens per pair
EPS = 1e-6
DC = DIM // 128               # 6 d-chunks
HC = HID // 128               # 24 hidden chunks

F32 = mybir.dt.float32
F32R = mybir.dt.float32r
BF16 = mybir.dt.bfloat16
AF = mybir.ActivationFunctionType
OP = mybir.AluOpType

# 128-aligned token tiles within a pair (for x/LN/O/out)
TOK_TILES = [(0, 128), (128, 128), (256, 128), (384, 10)]
# batch-aligned token tiles (for K/V in attention); tile 2*b + kt
V_TILES = [(0, 128), (128, 69), (197, 128), (325, 69)]
KT_TILES = [(0, 128), (128, 69)]  # (offset within batch, size)

DEBUG = bool(int(os.environ.get("BASSK_DEBUG", "0")))

_cached = None


def _build():
    nc = bacc.Bacc("TRN2", target_bir_lowering=False, debug=False)

    x_d = nc.dram_tensor("x", [T, DIM], F32, kind="ExternalInput").ap()
    w_d = {}
    for name, shape in [("Wq", [DIM, DIM]), ("Wk", [DIM, DIM]),
                        ("Wv", [DIM, DIM]), ("Wo", [DIM, DIM]),
                        ("W1", [DIM, HID]), ("W2", [HID, DIM]),
                        ("bq", [DIM]), ("bk", [DIM]), ("bv", [DIM]),
                        ("bo", [DIM]), ("b1", [HID]), ("b2", [DIM]),
                        ("g1", [DIM]), ("be1", [DIM]), ("g2", [DIM]),
                        ("be2", [DIM])]:
        w_d[name] = nc.dram_tensor(name, shape, F32, kind="ExternalInput").ap()
    out_d = nc.dram_tensor("out", [T, DIM], F32, kind="ExternalOutput").ap()

    dbg = {}
    if DEBUG:
        for name, shape in [("dq", [DIM, PT]), ("dk", [DIM, PT]),
                            ("dctx", [DIM, PT]), ("dx1", [512, DIM]),
                            ("dh", [HID, PT])]:
            dbg[name] = nc.dram_tensor(name, shape, BF16, kind="ExternalOutput").ap()

    with tile.TileContext(nc) as tc, ExitStack() as octx:
        persist = octx.enter_context(tc.tile_pool(name="persist", bufs=1))
        stage = octx.enter_context(tc.tile_pool(name="stage", bufs=2))
        wpool = octx.enter_context(tc.tile_pool(name="weights", bufs=1))

        # ---------- constants ----------
        st0 = stage.tile([128, DIM], F32, tag="stg")
        make_identity(nc, st0[:, 0:128])
        identb = persist.tile([128, 128], BF16)
        nc.vector.tensor_copy(identb[:], st0[:, 0:128])
        eps_sb = persist.tile([128, 1], F32)
        nc.vector.memset(eps_sb[:], EPS)
        ones_row = persist.tile([1, HD], BF16)
        nc.vector.memset(ones_row[:], 1.0)

        # per-partition bias layouts [128, nchunk] fp32
        bq_sb = persist.tile([128, DC], F32)
        nc.sync.dma_start(bq_sb[:], w_d["bq"].rearrange("(c p) -> p c", p=128))
        bk_sb = persist.tile([128, DC], F32)
        nc.sync.dma_start(bk_sb[:], w_d["bk"].rearrange("(c p) -> p c", p=128))
        b1_sb = persist.tile([128, HC], F32)
        nc.sync.dma_start(b1_sb[:], w_d["b1"].rearrange("(c p) -> p c", p=128))
        b2_sb = persist.tile([128, DC], F32)
        nc.sync.dma_start(b2_sb[:], w_d["b2"].rearrange("(c p) -> p c", p=128))

        # broadcast-[128, 768] bf16 tiles (staged fp32 -> cast)
        bcast = {}
        bias_names = ["bv", "bo", "g1", "be1", "g2", "be2"]
        for j, name in enumerate(bias_names):
            stb = stage.tile([128, DIM], F32, tag="stg")
            nc.sync.dma_start(stb[:],
                              w_d[name].unsqueeze(0).to_broadcast([128, DIM]))
            tb = persist.tile([128, DIM], BF16, name=f"bc_{name}")
            if j % 2 == 0:
                nc.vector.tensor_copy(tb[:], stb[:])
            else:
                nc.scalar.activation(tb[:], stb[:], AF.Copy, bias=0.0, scale=1.0)
            bcast[name] = tb

        # ---------- weights: stage fp32 + cast to bf16 ----------
        wq = wpool.tile([128, DC, DIM], BF16, name="wq")
        wk = wpool.tile([128, DC, DIM], BF16, name="wk")
        wv = wpool.tile([128, DC, DIM], BF16, name="wv")
        wo = wpool.tile([128, DC, DIM], BF16, name="wo")
        w1b = wpool.tile([128, DC, HID], BF16, name="w1b")
        w2b = wpool.tile([128, HC, DIM], BF16, name="w2b")

        _cast_idx = [0]

        def _cast(dst_ap, src_ap):
            # alternate engines so neither queue serializes
            if _cast_idx[0] % 2 == 0:
                nc.vector.tensor_copy(dst_ap, src_ap)
            else:
                nc.scalar.activation(dst_ap, src_ap, AF.Copy, bias=0.0, scale=1.0)
            _cast_idx[0] += 1

        def _load_square(dst, src):
            # [768, 768] fp32 -> [128, 6, 768] bf16, one chunk per stage tile
            for j in range(DC):
                stw = stage.tile([128, DIM], F32, tag="stg")
                nc.sync.dma_start(stw[:], src[j * 128:(j + 1) * 128, :])
                _cast(dst[:, j, :], stw[:])

        _load_square(wq, w_d["Wq"])
        _load_square(wk, w_d["Wk"])
        _load_square(wv, w_d["Wv"])
        _load_square(wo, w_d["Wo"])

        def _load_w1():
            # [768, 3072] -> [128, 6, 3072] bf16, quarter-chunks of 768
            for c in range(DC):
                for h2 in range(4):
                    stw = stage.tile([128, DIM], F32, tag="stg")
                    nc.sync.dma_start(
                        stw[:],
                        w_d["W1"][c * 128:(c + 1) * 128,
                                  h2 * DIM:(h2 + 1) * DIM])
                    _cast(w1b[:, c, h2 * DIM:(h2 + 1) * DIM], stw[:])

        def _load_w2():
            # [3072, 768] -> [128, 24, 768] bf16, one chunk per stage tile
            for j in range(HC):
                stw = stage.tile([128, DIM], F32, tag="stg")
                nc.sync.dma_start(stw[:], w_d["W2"][j * 128:(j + 1) * 128, :])
                _cast(w2b[:, j, :], stw[:])

        # ---------- per-pair activation pools ----------
        xpool = octx.enter_context(tc.tile_pool(name="xsb", bufs=2))
        tmp32 = octx.enter_context(tc.tile_pool(name="tmp32", bufs=4))
        trpool = octx.enter_context(tc.tile_pool(name="trT", bufs=2))
        qkpool = octx.enter_context(tc.tile_pool(name="qk", bufs=2))
        vpool = octx.enter_context(tc.tile_pool(name="v", bufs=1))
        etpool = octx.enter_context(tc.tile_pool(name="et", bufs=3))
        cxpool = octx.enter_context(tc.tile_pool(name="cx", bufs=1))
        smpool = octx.enter_context(tc.tile_pool(name="sm", bufs=2))
        x1pool = octx.enter_context(tc.tile_pool(name="x1", bufs=1))
        htpool = octx.enter_context(tc.tile_pool(name="ht", bufs=3))
        mopool = octx.enter_context(tc.tile_pool(name="mo", bufs=2))
        mfull = octx.enter_context(tc.tile_pool(name="mfull", bufs=1))
        lnpool = octx.enter_context(tc.tile_pool(name="ln", bufs=2))

        ps_mm = octx.enter_context(tc.tile_pool(name="psmm", bufs=2, space="PSUM"))

        def _emit_x_load(p):
            """DMA pair p's x into f32 landing tiles, cast to bf16 x_sb."""
            g0 = p * PT
            x_sb = xpool.tile([128, 4, DIM], BF16, tag="x")
            for i, (off, sz) in enumerate(TOK_TILES):
                land = tmp32.tile([128, DIM], F32, tag="t32")
                nc.sync.dma_start(land[0:sz, :], x_d[g0 + off:g0 + off + sz, :])
                nc.vector.tensor_copy(x_sb[0:sz, i, :], land[0:sz, :])
            return x_sb

        x_next = _emit_x_load(0)

        for p in range(NPAIR):
            g0 = p * PT
            x_sb = x_next

            # ---- prefetch next pair's x ----
            if p + 1 < NPAIR:
                x_next = _emit_x_load(p + 1)

            # ---- transpose x -> xT (d-major bf16) ----
            xT = trpool.tile([128, DC, PT], BF16, tag="trT")
            for i, (off, sz) in enumerate(TOK_TILES):
                for half in range(2):
                    pt = ps_mm.tile([128, 3, 128], BF16, tag="mm")
                    for cc in range(3):
                        c = half * 3 + cc
                        nc.tensor.transpose(pt[:, cc, 0:sz],
                                            x_sb[0:sz, i, c * 128:(c + 1) * 128],
                                            identb[0:sz, 0:sz])
                    nc.vector.tensor_copy(
                        xT[:, half * 3:half * 3 + 3, off:off + sz],
                        pt[:, :, 0:sz])

            # ---- Q/K projections (d-major bf16) ----
            qT = qkpool.tile([128, DC, PT], BF16, tag="qT")
            kT = qkpool.tile([128, DC, PT], BF16, tag="kT")
            for c in range(DC):
                for wt, bsb, dst in ((wq, bq_sb, qT), (wk, bk_sb, kT)):
                    pm = ps_mm.tile([128, PT], F32, tag="mm")
                    for kc in range(DC):
                        nc.tensor.matmul(pm[:, :],
                                         wt[:, kc, c * 128:(c + 1) * 128],
                                         xT[:, kc, :],
                                         start=(kc == 0), stop=(kc == DC - 1))
                    nc.scalar.activation(dst[:, c, :], pm[:, :], AF.Identity,
                                         bias=bsb[:, c:c + 1], scale=1.0)

            # ---- V projection (token-major, batch-aligned tiles) ----
            # 65th column holds ones so the ctx matmul also produces row-sums
            v_sb = vpool.tile([128, 4, NH, HD + 1], BF16, tag="v")
            nc.vector.memset(v_sb[:, :, :, HD:HD + 1], 1.0)
            for i, (off, sz) in enumerate(V_TILES):
                for s in range(2):
                    pm = ps_mm.tile([128, PT], F32, tag="mm")
                    for kc in range(DC):
                        nc.tensor.matmul(pm[0:sz, 0:384],
                                         xT[:, kc, off:off + sz],
                                         wv[:, kc, s * 384:(s + 1) * 384],
                                         start=(kc == 0), stop=(kc == DC - 1))
                    nc.vector.tensor_add(
                        v_sb[0:sz, i, 6 * s:6 * s + 6, 0:HD],
                        pm[0:sz, 0:384].rearrange("p (a b) -> p a b", a=6),
                        bcast["bv"][0:sz, s * 384:(s + 1) * 384]
                            .rearrange("p (a b) -> p a b", a=6))

            if p == 0:
                _load_w1()

            if DEBUG and p == 0:
                for c in range(DC):
                    nc.sync.dma_start(dbg["dq"][c * 128:(c + 1) * 128, :], qT[:, c, :])
                    nc.sync.dma_start(dbg["dk"][c * 128:(c + 1) * 128, :], kT[:, c, :])

            # ---- attention ----
            ctxT = cxpool.tile([128, DC, PT], BF16, tag="ctxT")
            with ExitStack() as actx:
                ps_at = actx.enter_context(
                    tc.tile_pool(name="psat", bufs=4, space="PSUM"))

                def _emit_scores(h):
                    """scores + exp for head h; returns (et_kt0, et_kt1)."""
                    hc, hp = h // 2, (h % 2) * 64
                    ets = []
                    for kt, (koff, ksz) in enumerate(KT_TILES):
                        psc = ps_at.tile([128, PT], F32, tag="at")
                        for b in range(2):
                            nc.tensor.matmul(
                                psc[0:ksz, b * S:(b + 1) * S],
                                kT[hp:hp + 64, hc, b * S + koff:b * S + koff + ksz],
                                qT[hp:hp + 64, hc, b * S:(b + 1) * S],
                                start=True, stop=True)
                        et = etpool.tile([128, PT], BF16, tag="et")
                        nc.scalar.activation(et[0:ksz, :], psc[0:ksz, :],
                                             AF.Exp, bias=0.0, scale=0.125)
                        ets.append(et)
                    return ets

                ets = _emit_scores(0)
                srf = None
                for h in range(NH):
                    hc, hp = h // 2, (h % 2) * 64
                    cur = ets
                    if h + 1 < NH:
                        ets = _emit_scores(h + 1)
                    # unnormalized ctx^T (row 64 = softmax denominators)
                    pc = ps_at.tile([128, PT], F32, tag="at")
                    for b in range(2):
                        for kt, (koff, ksz) in enumerate(KT_TILES):
                            nc.tensor.matmul(
                                pc[0:HD + 1, b * S:(b + 1) * S],
                                v_sb[0:ksz, 2 * b + kt, h, :],
                                cur[kt][0:ksz, b * S:(b + 1) * S],
                                start=(kt == 0), stop=(kt == 1))
                    nc.vector.tensor_copy(ctxT[hp:hp + 64, hc, :], pc[0:HD, :])
                    if hp == 0:
                        srf = smpool.tile([1, 2, PT], BF16, tag="srf")
                    nc.scalar.activation(srf[0:1, h % 2, :], pc[HD:HD + 1, :],
                                         AF.Copy, bias=0.0, scale=1.0)
                    if hp != 0:
                        # both heads of chunk hc done: broadcast their sums to
                        # 64 partitions each (contract-1 ones matmul), divide
                        pbc = ps_at.tile([128, PT], F32, tag="at")
                        for hh in range(2):
                            nc.tensor.matmul(pbc[hh * 64:hh * 64 + 64, :],
                                             ones_row[0:1, :],
                                             srf[0:1, hh, :],
                                             start=True, stop=True)
                        with nc.allow_low_precision(reason="softmax recip"):
                            nc.vector.reciprocal(pbc[:, :], pbc[:, :])
                        nc.vector.tensor_tensor(ctxT[:, hc, :], ctxT[:, hc, :],
                                                pbc[:, :], OP.mult)

            if p == 0:
                _load_w2()

            if DEBUG and p == 0:
                for c in range(DC):
                    nc.sync.dma_start(dbg["dctx"][c * 128:(c + 1) * 128, :],
                                      ctxT[:, c, :])

            # ---- O-projection + LN1 + residual -> x1 (bf16) ----
            x1 = x1pool.tile([128, 4, DIM], BF16, tag="x1")
            for i, (off, sz) in enumerate(TOK_TILES):
                ao = tmp32.tile([128, DIM], F32, tag="t32")
                for s in range(2):
                    pm = ps_mm.tile([128, PT], F32, tag="mm")
                    for c in range(DC):
                        nc.tensor.matmul(pm[0:sz, 0:384],
                                         ctxT[:, c, off:off + sz],
                                         wo[:, c, s * 384:(s + 1) * 384],
                                         start=(c == 0), stop=(c == DC - 1))
                    nc.vector.tensor_add(ao[0:sz, s * 384:(s + 1) * 384],
                                         pm[0:sz, 0:384],
                                         bcast["bo"][0:sz, s * 384:(s + 1) * 384])
                # LayerNorm 1 (stats in one bn_stats over 2x384 groups)
                st = lnpool.tile([128, 3, nc.vector.BN_STATS_DIM], F32, tag="st")
                for g in range(3):
                    nc.vector.bn_stats(st[0:sz, g, :],
                                       ao[0:sz, g * 256:(g + 1) * 256])
                mv = lnpool.tile([128, nc.vector.BN_AGGR_DIM], F32, tag="mv")
                nc.vector.bn_aggr(mv[0:sz, :], st[0:sz, :, :])
                rstd = lnpool.tile([128, 1], F32, tag="rstd")
                nc.scalar.activation(rstd[0:sz, :], mv[0:sz, 1:2], AF.Sqrt,
                                     bias=eps_sb[0:sz, :], scale=1.0)
                nc.vector.reciprocal(rstd[0:sz, :], rstd[0:sz, :])
                nmr = lnpool.tile([128, 1], F32, tag="nmr")
                nc.vector.tensor_scalar(nmr[0:sz, :], mv[0:sz, 0:1],
                                        rstd[0:sz, :], -1.0, OP.mult, OP.mult)
                tln = tmp32.tile([128, DIM], F32, tag="t32")
                nc.scalar.activation(tln[0:sz, :], ao[0:sz, :], AF.Identity,
                                     bias=nmr[0:sz, :], scale=rstd[0:sz, :])
                nc.vector.tensor_tensor(tln[0:sz, :], tln[0:sz, :],
                                        bcast["g1"][0:sz, :], OP.mult)
                xb = tmp32.tile([128, DIM], F32, tag="t32")
                nc.vector.tensor_add(xb[0:sz, :], x_sb[0:sz, i, :],
                                     bcast["be1"][0:sz, :])
                nc.vector.tensor_add(x1[0:sz, i, :], tln[0:sz, :], xb[0:sz, :])

            if DEBUG and p == 0:
                for i, (off, sz) in enumerate(TOK_TILES):
                    nc.sync.dma_start(dbg["dx1"][128 * i:128 * i + sz, :],
                                      x1[0:sz, i, :])

            # ---- transpose x1 -> x1T (d-major bf16) ----
            x1T = trpool.tile([128, DC, PT], BF16, tag="trT")
            for i, (off, sz) in enumerate(TOK_TILES):
                for half in range(2):
                    pt = ps_mm.tile([128, 3, 128], BF16, tag="mm")
                    for cc in range(3):
                        c = half * 3 + cc
                        nc.tensor.transpose(pt[:, cc, 0:sz],
                                            x1[0:sz, i, c * 128:(c + 1) * 128],
                                            identb[0:sz, 0:sz])
                    nc.vector.tensor_copy(
                        x1T[:, half * 3:half * 3 + 3, off:off + sz],
                        pt[:, :, 0:sz])

            # ---- MLP: interleaved MLP1 (gelu) and MLP2 (6-bank acc) ----
            with ExitStack() as mctx:
                ps_ac = mctx.enter_context(
                    tc.tile_pool(name="psac", bufs=6, space="PSUM"))
                pacs = [ps_ac.tile([128, PT], F32, tag="ac", name=f"pac{c}")
                        for c in range(DC)]

                def _emit_mlp1(hcx):
                    pm = ps_mm.tile([128, PT], F32, tag="mm")
                    for kc in range(DC):
                        nc.tensor.matmul(pm[:, :],
                                         w1b[:, kc, hcx * 128:(hcx + 1) * 128],
                                         x1T[:, kc, :],
                                         start=(kc == 0), stop=(kc == DC - 1))
                    hTc = htpool.tile([128, PT], BF16, tag="hT")
                    nc.scalar.activation(hTc[:, :], pm[:, :], AF.Gelu,
                                         bias=b1_sb[:, hcx:hcx + 1], scale=1.0)
                    return hTc

                hT_cur = _emit_mlp1(0)
                for hcx in range(HC):
                    hT_use = hT_cur
                    if hcx + 1 < HC:
                        hT_cur = _emit_mlp1(hcx + 1)
                    for c in range(DC):
                        nc.tensor.matmul(pacs[c][:, :],
                                         w2b[:, hcx, c * 128:(c + 1) * 128],
                                         hT_use[:, :],
                                         start=(hcx == 0), stop=(hcx == HC - 1))
                    if DEBUG and p == 0:
                        nc.sync.dma_start(
                            dbg["dh"][hcx * 128:(hcx + 1) * 128, :], hT_use[:, :])

                # ---- evict mlp_out^T (+b2), transpose back to token-major ----
                mo = mfull.tile([128, 4, DIM], BF16, tag="mo")
                for c in range(DC):
                    moTc = mopool.tile([128, PT], BF16, tag="moT")
                    nc.scalar.activation(moTc[:, :], pacs[c][:, :], AF.Identity,
                                         bias=b2_sb[:, c:c + 1], scale=1.0)
                    ptc = ps_mm.tile([128, 4, 128], BF16, tag="mm")
                    for i, (off, sz) in enumerate(TOK_TILES):
                        nc.tensor.transpose(ptc[0:sz, i, :],
                                            moTc[:, off:off + sz],
                                            identb[:, :])
                    nc.vector.tensor_copy(mo[:, 0:3, c * 128:(c + 1) * 128],
                                          ptc[:, 0:3, :])
                    nc.vector.tensor_copy(mo[0:10, 3, c * 128:(c + 1) * 128],
                                          ptc[0:10, 3, :])

            # ---- LN2 + residual -> out ----
            for i, (off, sz) in enumerate(TOK_TILES):
                st = lnpool.tile([128, 3, nc.vector.BN_STATS_DIM], F32, tag="st")
                for g in range(3):
                    nc.vector.bn_stats(st[0:sz, g, :],
                                       mo[0:sz, i, g * 256:(g + 1) * 256])
                mv = lnpool.tile([128, nc.vector.BN_AGGR_DIM], F32, tag="mv")
                nc.vector.bn_aggr(mv[0:sz, :], st[0:sz, :, :])
                rstd = lnpool.tile([128, 1], F32, tag="rstd")
                nc.scalar.activation(rstd[0:sz, :], mv[0:sz, 1:2], AF.Sqrt,
                                     bias=eps_sb[0:sz, :], scale=1.0)
                nc.vector.reciprocal(rstd[0:sz, :], rstd[0:sz, :])
                nmr = lnpool.tile([128, 1], F32, tag="nmr")
                nc.vector.tensor_scalar(nmr[0:sz, :], mv[0:sz, 0:1],
                                        rstd[0:sz, :], -1.0, OP.mult, OP.mult)
                tln = tmp32.tile([128, DIM], F32, tag="t32")
                nc.scalar.activation(tln[0:sz, :], mo[0:sz, i, :], AF.Identity,
                                     bias=nmr[0:sz, :], scale=rstd[0:sz, :])
                nc.vector.tensor_tensor(tln[0:sz, :], tln[0:sz, :],
                                        bcast["g2"][0:sz, :], OP.mult)
                xb = tmp32.tile([128, DIM], F32, tag="t32")
                nc.vector.tensor_add(xb[0:sz, :], x1[0:sz, i, :],
                                     bcast["be2"][0:sz, :])
                ot = tmp32.tile([128, DIM], F32, tag="t32")
                nc.vector.tensor_add(ot[0:sz, :], tln[0:sz, :], xb[0:sz, :])
                nc.sync.dma_start(out_d[g0 + off:g0 + off + sz, :], ot[0:sz, :])

    nc.compile()
    return nc


def kernel(x, Wq, bq, Wk, bk, Wv, bv, Wo, bo, W1, b1, W2, b2, g1, be1, g2, be2):
    global _cached
    if _cached is None:
        _cached = _build()
    nc = _cached

    weights = dict(Wq=Wq, bq=bq, Wk=Wk, bk=bk, Wv=Wv, bv=bv, Wo=Wo, bo=bo,
                   W1=W1, b1=b1, W2=W2, b2=b2, g1=g1, be1=be1, g2=g2, be2=be2)
    weights = {k: np.ascontiguousarray(v, dtype=np.float32) for k, v in weights.items()}
    x = np.asarray(x, dtype=np.float32)

    in_maps = []
    for c in range(N_CORES):
        xc = np.ascontiguousarray(
            x[c * BPC:(c + 1) * BPC].reshape(T, DIM))
        in_maps.append({"x": xc, **weights})

    res = run_bass_kernel_spmd(nc, in_maps, core_ids=list(range(N_CORES)),
                               trace=bool(int(os.environ.get("BASSK_TRACE", "0"))))
    kernel._last_res = res
    out = np.concatenate(
        [res.results[c]["out"].reshape(BPC, S, DIM) for c in range(N_CORES)], axis=0)
    return out.astype(np.float32)


# revision 20
# speedup vs baseline: 1.5669x; 1.0583x over previous
"""Trainium2 Bass kernel for a ViT-Base transformer encoder block.

Input x: [64, 197, 768] fp32 + weights. Data-parallel over batch across 8
NeuronCores (8 batches/core = 1576 tokens/core). Single fused pass per
batch-pair (394 tokens), 4 pairs per core:

  x -> (bf16) xT -> Q/K/V projections (bf16 matmuls) -> attention with
  per-batch moving dim 197 (bf16), row-sums accumulated into a shared PSUM
  via ones-matmuls, one reciprocal for all heads, PE-broadcast of the
  per-(head,token) reciprocals via a 0/1 selection matrix -> O-projection ->
  LN1+residual -> x1 (bf16, kept in SBUF) -> MLP (bf16, W1/W2 resident,
  interleaved MLP1/MLP2 with 6-bank PSUM accumulation) -> transpose back ->
  LN2+residual -> out.

All weights are converted to bf16 on-chip once (staged fp32 DMA + cast).
"""
import os
import sys

sys.path.insert(0, "/opt/trn_rl_repo")

import numpy as np
from contextlib import ExitStack

import concourse.bass as bass
import concourse.tile as tile
from concourse import bacc, mybir
from concourse.bass_utils import run_bass_kernel_spmd
from concourse.masks import make_identity

DIM, NH, HD, HID = 768, 12, 64, 3072
S = 197
B = 64
N_CORES = 8
BPC = B // N_CORES            # 8 batches per core
T = BPC * S                   # 1576 tokens per core
NPAIR = BPC // 2              # 4 batch pairs per core
PT = 2 * S                    # 394 tokens per pair
EPS = 1e-6
DC = DIM // 128               # 6 d-chunks
HC = HID // 128               # 24 hidden chunks

F32 = mybir.dt.float32
F32R = mybir.dt.float32r
BF16 = mybir.dt.bfloat16
AF = mybir.ActivationFunctionType
OP = mybir.AluOpType

# 128-aligned token tiles within a pair (for x/LN/O/out)
TOK_TILES = [(0, 128), (128, 128), (256, 128), (384, 10)]
# batch-aligned token tiles (for K/V in attention); tile 2*b + kt
V_TILES = [(0, 128), (128, 69), (197, 128), (325, 69)]
KT_TILES = [(0, 128), (128, 69)]  # (offset within batch, size)

DEBUG = bool(int(os.environ.get("BASSK_DEBUG", "0")))

_cached = None


def _build():
    nc = bacc.Bacc("TRN2", target_bir_lowering=False, debug=False)

    x_d = nc.dram_tensor("x", [T, DIM], F32, kind="ExternalInput").ap()
    w_d = {}
    for name, shape in [("Wq", [DIM, DIM]), ("Wk", [DIM, DIM]),
                        ("Wv", [DIM, DIM]), ("Wo", [DIM, DIM]),
                        ("W1", [DIM, HID]), ("W2", [HID, DIM]),
                        ("bq", [DIM]), ("bk", [DIM]), ("bv", [DIM]),
                        ("bo", [DIM]), ("b1", [HID]), ("b2", [DIM]),
                        ("g1", [DIM]), ("be1", [DIM]), ("g2", [DIM]),
                        ("be2", [DIM])]:
        w_d[name] = nc.dram_tensor(name, shape, F32, kind="ExternalInput").ap()
    out_d = nc.dram_tensor("out", [T, DIM], F32, kind="ExternalOutput").ap()

    dbg = {}
    if DEBUG:
        for name, shape in [("dq", [DIM, PT]), ("dk", [DIM, PT]),
                            ("dctx", [DIM, PT]), ("dx1", [512, DIM]),
                            ("dh", [HID, PT])]:
            dbg[name] = nc.dram_tensor(name, shape, BF16, kind="ExternalOutput").ap()

    with tile.TileContext(nc) as tc, ExitStack() as octx:
        persist = octx.enter_context(tc.tile_pool(name="persist", bufs=1))
        stage = octx.enter_context(tc.tile_pool(name="stage", bufs=2))
        wpool = octx.enter_context(tc.tile_pool(name="weights", bufs=1))

        # ---------- constants ----------
        st0 = stage.tile([128, DIM], F32, tag="stg")
        make_identity(nc, st0[:, 0:128])
        identb = persist.tile([128, 128], BF16)
        nc.vector.tensor_copy(identb[:], st0[:, 0:128])
        eps_sb = persist.tile([128, 1], F32)
        nc.vector.memset(eps_sb[:], EPS)
        ones_row = persist.tile([1, HD], BF16)
        nc.vector.memset(ones_row[:], 1.0)

        # per-partition bias layouts [128, nchunk] fp32
        bq_sb = persist.tile([128, DC], F32)
        nc.sync.dma_start(bq_sb[:], w_d["bq"].rearrange("(c p) -> p c", p=128))
        bk_sb = persist.tile([128, DC], F32)
        nc.sync.dma_start(bk_sb[:], w_d["bk"].rearrange("(c p) -> p c", p=128))
        b1_sb = persist.tile([128, HC], F32)
        nc.sync.dma_start(b1_sb[:], w_d["b1"].rearrange("(c p) -> p c", p=128))
        b2_sb = persist.tile([128, DC], F32)
        nc.sync.dma_start(b2_sb[:], w_d["b2"].rearrange("(c p) -> p c", p=128))

        # broadcast-[128, 768] bf16 tiles (staged fp32 -> cast)
        bcast = {}
        bias_names = ["bv", "bo", "g1", "be1", "g2", "be2"]
        for j, name in enumerate(bias_names):
            stb = stage.tile([128, DIM], F32, tag="stg")
            nc.sync.dma_start(stb[:],
                              w_d[name].unsqueeze(0).to_broadcast([128, DIM]))
            tb = persist.tile([128, DIM], BF16, name=f"bc_{name}")
            if j % 2 == 0:
                nc.vector.tensor_copy(tb[:], stb[:])
            else:
                nc.scalar.activation(tb[:], stb[:], AF.Copy, bias=0.0, scale=1.0)
            bcast[name] = tb

        # ---------- weights: stage fp32 + cast to bf16 ----------
        wq = wpool.tile([128, DC, DIM], BF16, name="wq")
        wk = wpool.tile([128, DC, DIM], BF16, name="wk")
        wv = wpool.tile([128, DC, DIM], BF16, name="wv")
        wo = wpool.tile([128, DC, DIM], BF16, name="wo")
        w1b = wpool.tile([128, DC, HID], BF16, name="w1b")
        w2b = wpool.tile([128, HC, DIM], BF16, name="w2b")

        _cast_idx = [0]

        def _cast(dst_ap, src_ap):
            # alternate engines so neither queue serializes
            if _cast_idx[0] % 2 == 0:
                nc.vector.tensor_copy(dst_ap, src_ap)
            else:
                nc.scalar.activation(dst_ap, src_ap, AF.Copy, bias=0.0, scale=1.0)
            _cast_idx[0] += 1

        def _load_square(dst, src):
            # [768, 768] fp32 -> [128, 6, 768] bf16, one chunk per stage tile
            for j in range(DC):
                stw = stage.tile([128, DIM], F32, tag="stg")
                nc.sync.dma_start(stw[:], src[j * 128:(j + 1) * 128, :])
                _cast(dst[:, j, :], stw[:])

        _load_square(wq, w_d["Wq"])
        _load_square(wk, w_d["Wk"])
        _load_square(wv, w_d["Wv"])
        _load_square(wo, w_d["Wo"])

        def _load_w1():
            # [768, 3072] -> [128, 6, 3072] bf16, quarter-chunks of 768
            for c in range(DC):
                for h2 in range(4):
                    stw = stage.tile([128, DIM], F32, tag="stg")
                    nc.sync.dma_start(
                        stw[:],
                        w_d["W1"][c * 128:(c + 1) * 128,
                                  h2 * DIM:(h2 + 1) * DIM])
                    _cast(w1b[:, c, h2 * DIM:(h2 + 1) * DIM], stw[:])

        def _load_w2():
            # [3072, 768] -> [128, 24, 768] bf16, one chunk per stage tile
            for j in range(HC):
                stw = stage.tile([128, DIM], F32, tag="stg")
                nc.sync.dma_start(stw[:], w_d["W2"][j * 128:(j + 1) * 128, :])
                _cast(w2b[:, j, :], stw[:])

        # ---------- per-pair activation pools ----------
        xpool = octx.enter_context(tc.tile_pool(name="xsb", bufs=2))
        tmp32 = octx.enter_context(tc.tile_pool(name="tmp32", bufs=4))
        trpool = octx.enter_context(tc.tile_pool(name="trT", bufs=2))
        qkpool = octx.enter_context(tc.tile_pool(name="qk", bufs=2))
        vpool = octx.enter_context(tc.tile_pool(name="v", bufs=1))
        etpool = octx.enter_context(tc.tile_pool(name="et", bufs=3))
        cxpool = octx.enter_context(tc.tile_pool(name="cx", bufs=1))
        smpool = octx.enter_context(tc.tile_pool(name="sm", bufs=2))
        x1pool = octx.enter_context(tc.tile_pool(name="x1", bufs=1))
        htpool = octx.enter_context(tc.tile_pool(name="ht", bufs=3))
        mopool = octx.enter_context(tc.tile_pool(name="mo", bufs=2))
        mfull = octx.enter_context(tc.tile_pool(name="mfull", bufs=1))
        lnpool = octx.enter_context(tc.tile_pool(name="ln", bufs=2))

        ps_mm = octx.enter_context(tc.tile_pool(name="psmm", bufs=2, space="PSUM"))

        def _emit_x_load(p):
            """DMA pair p's x into f32 landing tiles, cast to bf16 x_sb."""
            g0 = p * PT
            x_sb = xpool.tile([128, 4, DIM], BF16, tag="x")
            for i, (off, sz) in enumerate(TOK_TILES):
                land = tmp32.tile([128, DIM], F32, tag="t32")
                nc.sync.dma_start(land[0:sz, :], x_d[g0 + off:g0 + off + sz, :])
                nc.vector.tensor_copy(x_sb[0:sz, i, :], land[0:sz, :])
            return x_sb

        x_next = _emit_x_load(0)

        def _emit_tail(tp, pacs, psac_ctx, x1t):
            """MLP2 eviction, transpose back, LN2 + residual, out DMA for
            pair tp. Emitted after pair tp+1's transpose/QKV phases so the
            PE never waits on this (DVE/Act-heavy) tail at pair boundaries."""
            tg0 = tp * PT
            mo = mfull.tile([128, 4, DIM], BF16, tag="mo")
            for c in range(DC):
                moTc = mopool.tile([128, PT], BF16, tag="moT")
                nc.scalar.activation(moTc[:, :], pacs[c][:, :], AF.Identity,
                                     bias=b2_sb[:, c:c + 1], scale=1.0)
                ptc = ps_mm.tile([128, 4, 128], BF16, tag="mm")
                for i, (off, sz) in enumerate(TOK_TILES):
                    nc.tensor.transpose(ptc[0:sz, i, :],
                                        moTc[:, off:off + sz],
                                        identb[:, :])
                nc.vector.tensor_copy(mo[:, 0:3, c * 128:(c + 1) * 128],
                                      ptc[:, 0:3, :])
                nc.vector.tensor_copy(mo[0:10, 3, c * 128:(c + 1) * 128],
                                      ptc[0:10, 3, :])
            psac_ctx.close()

            # LN2 + residual -> out
            for i, (off, sz) in enumerate(TOK_TILES):
                st = lnpool.tile([128, 3, nc.vector.BN_STATS_DIM], F32, tag="st")
                for g in range(3):
                    nc.vector.bn_stats(st[0:sz, g, :],
                                       mo[0:sz, i, g * 256:(g + 1) * 256])
                mv = lnpool.tile([128, nc.vector.BN_AGGR_DIM], F32, tag="mv")
                nc.vector.bn_aggr(mv[0:sz, :], st[0:sz, :, :])
                rstd = lnpool.tile([128, 1], F32, tag="rstd")
                nc.scalar.activation(rstd[0:sz, :], mv[0:sz, 1:2], AF.Sqrt,
                                     bias=eps_sb[0:sz, :], scale=1.0)
                nc.vector.reciprocal(rstd[0:sz, :], rstd[0:sz, :])
                nmr = lnpool.tile([128, 1], F32, tag="nmr")
                nc.vector.tensor_scalar(nmr[0:sz, :], mv[0:sz, 0:1],
                                        rstd[0:sz, :], -1.0, OP.mult, OP.mult)
                tln = tmp32.tile([128, DIM], F32, tag="t32")
                nc.scalar.activation(tln[0:sz, :], mo[0:sz, i, :], AF.Identity,
                                     bias=nmr[0:sz, :], scale=rstd[0:sz, :])
                nc.vector.tensor_tensor(tln[0:sz, :], tln[0:sz, :],
                                        bcast["g2"][0:sz, :], OP.mult)
                xb = tmp32.tile([128, DIM], F32, tag="t32")
                nc.gpsimd.tensor_add(xb[0:sz, :], x1t[0:sz, i, :],
                                     bcast["be2"][0:sz, :])
                ot = tmp32.tile([128, DIM], F32, tag="t32")
                nc.vector.tensor_add(ot[0:sz, :], tln[0:sz, :], xb[0:sz, :])
                nc.sync.dma_start(out_d[tg0 + off:tg0 + off + sz, :], ot[0:sz, :])

        pending = None

        for p in range(NPAIR):
            g0 = p * PT
            x_sb = x_next

            # ---- prefetch next pair's x ----
            if p + 1 < NPAIR:
                x_next = _emit_x_load(p + 1)

            # ---- transpose x -> xT (d-major bf16) ----
            xT = trpool.tile([128, DC, PT], BF16, tag="trT")
            for i, (off, sz) in enumerate(TOK_TILES):
                for half in range(2):
                    pt = ps_mm.tile([128, 3, 128], BF16, tag="mm")
                    for cc in range(3):
                        c = half * 3 + cc
                        nc.tensor.transpose(pt[:, cc, 0:sz],
                                            x_sb[0:sz, i, c * 128:(c + 1) * 128],
                                            identb[0:sz, 0:sz])
                    nc.vector.tensor_copy(
                        xT[:, half * 3:half * 3 + 3, off:off + sz],
                        pt[:, :, 0:sz])

            # ---- Q/K projections (d-major bf16) ----
            qT = qkpool.tile([128, DC, PT], BF16, tag="qT")
            kT = qkpool.tile([128, DC, PT], BF16, tag="kT")
            for c in range(DC):
                for wt, bsb, dst in ((wq, bq_sb, qT), (wk, bk_sb, kT)):
                    pm = ps_mm.tile([128, PT], F32, tag="mm")
                    for kc in range(DC):
                        nc.tensor.matmul(pm[:, :],
                                         wt[:, kc, c * 128:(c + 1) * 128],
                                         xT[:, kc, :],
                                         start=(kc == 0), stop=(kc == DC - 1))
                    nc.scalar.activation(dst[:, c, :], pm[:, :], AF.Identity,
                                         bias=bsb[:, c:c + 1], scale=1.0)

            # ---- V projection (token-major, batch-aligned tiles) ----
            # 65th column holds ones so the ctx matmul also produces row-sums
            v_sb = vpool.tile([128, 4, NH, HD + 1], BF16, tag="v")
            nc.vector.memset(v_sb[:, :, :, HD:HD + 1], 1.0)
            for i, (off, sz) in enumerate(V_TILES):
                for s in range(2):
                    pm = ps_mm.tile([128, PT], F32, tag="mm")
                    for kc in range(DC):
                        nc.tensor.matmul(pm[0:sz, 0:384],
                                         xT[:, kc, off:off + sz],
                                         wv[:, kc, s * 384:(s + 1) * 384],
                                         start=(kc == 0), stop=(kc == DC - 1))
                    nc.vector.tensor_add(
                        v_sb[0:sz, i, 6 * s:6 * s + 6, 0:HD],
                        pm[0:sz, 0:384].rearrange("p (a b) -> p a b", a=6),
                        bcast["bv"][0:sz, s * 384:(s + 1) * 384]
                            .rearrange("p (a b) -> p a b", a=6))

            if p == 0:
                _load_w1()

            # ---- previous pair's MLP eviction + LN2 tail ----
            if pending is not None:
                _emit_tail(*pending)
                pending = None

            if DEBUG and p == 0:
                for c in range(DC):
                    nc.sync.dma_start(dbg["dq"][c * 128:(c + 1) * 128, :], qT[:, c, :])
                    nc.sync.dma_start(dbg["dk"][c * 128:(c + 1) * 128, :], kT[:, c, :])

            # ---- attention ----
            ctxT = cxpool.tile([128, DC, PT], BF16, tag="ctxT")
            with ExitStack() as actx:
                ps_at = actx.enter_context(
                    tc.tile_pool(name="psat", bufs=4, space="PSUM"))

                def _emit_scores(h):
                    """scores + exp for head h; returns (et_kt0, et_kt1)."""
                    hc, hp = h // 2, (h % 2) * 64
                    ets = []
                    for kt, (koff, ksz) in enumerate(KT_TILES):
                        psc = ps_at.tile([128, PT], F32, tag="at")
                        for b in range(2):
                            nc.tensor.matmul(
                                psc[0:ksz, b * S:(b + 1) * S],
                                kT[hp:hp + 64, hc, b * S + koff:b * S + koff + ksz],
                                qT[hp:hp + 64, hc, b * S:(b + 1) * S],
                                start=True, stop=True)
                        et = etpool.tile([128, PT], BF16, tag="et")
                        nc.scalar.activation(et[0:ksz, :], psc[0:ksz, :],
                                             AF.Exp, bias=0.0, scale=0.125)
                        ets.append(et)
                    return ets

                ets = _emit_scores(0)
                srf = None
                for h in range(NH):
                    hc, hp = h // 2, (h % 2) * 64
                    cur = ets
                    if h + 1 < NH:
                        ets = _emit_scores(h + 1)
                    # unnormalized ctx^T (row 64 = softmax denominators)
                    pc = ps_at.tile([128, PT], F32, tag="at")
                    for b in range(2):
                        for kt, (koff, ksz) in enumerate(KT_TILES):
                            nc.tensor.matmul(
                                pc[0:HD + 1, b * S:(b + 1) * S],
                                v_sb[0:ksz, 2 * b + kt, h, :],
                                cur[kt][0:ksz, b * S:(b + 1) * S],
                                start=(kt == 0), stop=(kt == 1))
                    nc.vector.tensor_copy(ctxT[hp:hp + 64, hc, :], pc[0:HD, :])
                    if hp == 0:
                        srf = smpool.tile([1, 2, PT], BF16, tag="srf")
                    nc.scalar.activation(srf[0:1, h % 2, :], pc[HD:HD + 1, :],
                                         AF.Copy, bias=0.0, scale=1.0)
                    if hp != 0:
                        # both heads of chunk hc done: broadcast their sums to
                        # 64 partitions each (contract-1 ones matmul), divide
                        pbc = ps_at.tile([128, PT], F32, tag="at")
                        for hh in range(2):
                            nc.tensor.matmul(pbc[hh * 64:hh * 64 + 64, :],
                                             ones_row[0:1, :],
                                             srf[0:1, hh, :],
                                             start=True, stop=True)
                        nc.vector.reciprocal_approx_fast(pbc[:, :], pbc[:, :])
                        nc.vector.tensor_tensor(ctxT[:, hc, :], ctxT[:, hc, :],
                                                pbc[:, :], OP.mult)

            if p == 0:
                _load_w2()

            if DEBUG and p == 0:
                for c in range(DC):
                    nc.sync.dma_start(dbg["dctx"][c * 128:(c + 1) * 128, :],
                                      ctxT[:, c, :])

            # ---- O-projection + LN1 + residual -> x1 (bf16) ----
            x1 = x1pool.tile([128, 4, DIM], BF16, tag="x1")
            for i, (off, sz) in enumerate(TOK_TILES):
                ao = tmp32.tile([128, DIM], F32, tag="t32")
                for s in range(2):
                    pm = ps_mm.tile([128, PT], F32, tag="mm")
                    for c in range(DC):
                        nc.tensor.matmul(pm[0:sz, 0:384],
                                         ctxT[:, c, off:off + sz],
                                         wo[:, c, s * 384:(s + 1) * 384],
                                         start=(c == 0), stop=(c == DC - 1))
                    nc.vector.tensor_add(ao[0:sz, s * 384:(s + 1) * 384],
                                         pm[0:sz, 0:384],
                                         bcast["bo"][0:sz, s * 384:(s + 1) * 384])
                # LayerNorm 1 (stats in one bn_stats over 2x384 groups)
                st = lnpool.tile([128, 3, nc.vector.BN_STATS_DIM], F32, tag="st")
                for g in range(3):
                    nc.vector.bn_stats(st[0:sz, g, :],
                                       ao[0:sz, g * 256:(g + 1) * 256])
                mv = lnpool.tile([128, nc.vector.BN_AGGR_DIM], F32, tag="mv")
                nc.vector.bn_aggr(mv[0:sz, :], st[0:sz, :, :])
                rstd = lnpool.tile([128, 1], F32, tag="rstd")
                nc.scalar.activation(rstd[0:sz, :], mv[0:sz, 1:2], AF.Sqrt,
                                     bias=eps_sb[0:sz, :], scale=1.0)
                nc.vector.reciprocal(rstd[0:sz, :], rstd[0:sz, :])
                nmr = lnpool.tile([128, 1], F32, tag="nmr")
                nc.vector.tensor_scalar(nmr[0:sz, :], mv[0:sz, 0:1],
                                        rstd[0:sz, :], -1.0, OP.mult, OP.mult)
                tln = tmp32.tile([128, DIM], F32, tag="t32")
                nc.scalar.activation(tln[0:sz, :], ao[0:sz, :], AF.Identity,
                                     bias=nmr[0:sz, :], scale=rstd[0:sz, :])
                nc.vector.tensor_tensor(tln[0:sz, :], tln[0:sz, :],
                                        bcast["g1"][0:sz, :], OP.mult)
                xb = tmp32.tile([128, DIM], F32, tag="t32")
                nc.gpsimd.tensor_add(xb[0:sz, :], x_sb[0:sz, i, :],
                                     bcast["be1"][0:sz, :])
                nc.vector.tensor_add(x1[0:sz, i, :], tln[0:sz, :], xb[0:sz, :])

            if DEBUG and p == 0:
                for i, (off, sz) in enumerate(TOK_TILES):
                    nc.sync.dma_start(dbg["dx1"][128 * i:128 * i + sz, :],
                                      x1[0:sz, i, :])

            # ---- transpose x1 -> x1T (d-major bf16) ----
            x1T = trpool.tile([128, DC, PT], BF16, tag="trT")
            for i, (off, sz) in enumerate(TOK_TILES):
                for half in range(2):
                    pt = ps_mm.tile([128, 3, 128], BF16, tag="mm")
                    for cc in range(3):
                        c = half * 3 + cc
                        nc.tensor.transpose(pt[:, cc, 0:sz],
                                            x1[0:sz, i, c * 128:(c + 1) * 128],
                                            identb[0:sz, 0:sz])
                    nc.vector.tensor_copy(
                        x1T[:, half * 3:half * 3 + 3, off:off + sz],
                        pt[:, :, 0:sz])

            # ---- MLP: interleaved MLP1 (gelu) and MLP2 (6-bank acc) ----
            psac_ctx = ExitStack()
            ps_ac = psac_ctx.enter_context(
                tc.tile_pool(name=f"psac{p}", bufs=6, space="PSUM"))
            pacs = [ps_ac.tile([128, PT], F32, tag="ac", name=f"pac{c}")
                    for c in range(DC)]

            def _emit_mlp1(hcx):
                pm = ps_mm.tile([128, PT], F32, tag="mm")
                for kc in range(DC):
                    nc.tensor.matmul(pm[:, :],
                                     w1b[:, kc, hcx * 128:(hcx + 1) * 128],
                                     x1T[:, kc, :],
                                     start=(kc == 0), stop=(kc == DC - 1))
                hTc = htpool.tile([128, PT], BF16, tag="hT")
                nc.scalar.activation(hTc[:, :], pm[:, :], AF.Gelu,
                                     bias=b1_sb[:, hcx:hcx + 1], scale=1.0)
                return hTc

            hT_cur = _emit_mlp1(0)
            for hcx in range(HC):
                hT_use = hT_cur
                if hcx + 1 < HC:
                    hT_cur = _emit_mlp1(hcx + 1)
                for c in range(DC):
                    nc.tensor.matmul(pacs[c][:, :],
                                     w2b[:, hcx, c * 128:(c + 1) * 128],
                                     hT_use[:, :],
                                     start=(hcx == 0), stop=(hcx == HC - 1))
                if DEBUG and p == 0:
                    nc.sync.dma_start(
                        dbg["dh"][hcx * 128:(hcx + 1) * 128, :], hT_use[:, :])

            pending = (p, pacs, psac_ctx, x1)

        # tail of the last pair
        _emit_tail(*pending)

    nc.compile()
    return nc


def kernel(x, Wq, bq, Wk, bk, Wv, bv, Wo, bo, W1, b1, W2, b2, g1, be1, g2, be2):
    global _cached
    if _cached is None:
        _cached = _build()
    nc = _cached

    weights = dict(Wq=Wq, bq=bq, Wk=Wk, bk=bk, Wv=Wv, bv=bv, Wo=Wo, bo=bo,
                   W1=W1, b1=b1, W2=W2, b2=b2, g1=g1, be1=be1, g2=g2, be2=be2)
    weights = {k: np.ascontiguousarray(v, dtype=np.float32) for k, v in weights.items()}
    x = np.asarray(x, dtype=np.float32)

    in_maps = []
    for c in range(N_CORES):
        xc = np.ascontiguousarray(
            x[c * BPC:(c + 1) * BPC].reshape(T, DIM))
        in_maps.append({"x": xc, **weights})

    res = run_bass_kernel_spmd(nc, in_maps, core_ids=list(range(N_CORES)),
                               trace=bool(int(os.environ.get("BASSK_TRACE", "0"))))
    kernel._last_res = res
    out = np.concatenate(
        [res.results[c]["out"].reshape(BPC, S, DIM) for c in range(N_CORES)], axis=0)
    return out.astype(np.float32)


# revision 25
# speedup vs baseline: 1.6365x; 1.0444x over previous
"""Trainium2 Bass kernel for a ViT-Base transformer encoder block.

Input x: [64, 197, 768] fp32 + weights. Data-parallel over batch across 8
NeuronCores (8 batches/core = 1576 tokens/core). Single fused pass per
batch-pair (394 tokens), 4 pairs per core:

  x -> (bf16) xT -> Q/K/V projections (bf16 matmuls) -> attention with
  per-batch moving dim 197 (bf16), row-sums accumulated into a shared PSUM
  via ones-matmuls, one reciprocal for all heads, PE-broadcast of the
  per-(head,token) reciprocals via a 0/1 selection matrix -> O-projection ->
  LN1+residual -> x1 (bf16, kept in SBUF) -> MLP (bf16, W1/W2 resident,
  interleaved MLP1/MLP2 with 6-bank PSUM accumulation) -> transpose back ->
  LN2+residual -> out.

All weights are converted to bf16 on-chip once (staged fp32 DMA + cast).
"""
import os
import sys

sys.path.insert(0, "/opt/trn_rl_repo")

import numpy as np
from contextlib import ExitStack

import concourse.bass as bass
import concourse.tile as tile
from concourse import bacc, mybir
from concourse.bass_utils import run_bass_kernel_spmd
from concourse.masks import make_identity

DIM, NH, HD, HID = 768, 12, 64, 3072
S = 197
B = 64
N_CORES = 8
BPC = B // N_CORES            # 8 batches per core
T = BPC * S                   # 1576 tokens per core
NPAIR = BPC // 2              # 4 batch pairs per core
PT = 2 * S                    # 394 tokens per pair
EPS = 1e-6
DC = DIM // 128               # 6 d-chunks
HC = HID // 128               # 24 hidden chunks

F32 = mybir.dt.float32
F32R = mybir.dt.float32r
BF16 = mybir.dt.bfloat16
AF = mybir.ActivationFunctionType
OP = mybir.AluOpType

# 128-aligned token tiles within a pair (for x/LN/O/out)
TOK_TILES = [(0, 128), (128, 128), (256, 128), (384, 10)]
# batch-aligned token tiles (for K/V in attention); tile 2*b + kt
V_TILES = [(0, 128), (128, 69), (197, 128), (325, 69)]
KT_TILES = [(0, 128), (128, 69)]  # (offset within batch, size)

DEBUG = bool(int(os.environ.get("BASSK_DEBUG", "0")))

_cached = None


def _build():
    nc = bacc.Bacc("TRN2", target_bir_lowering=False, debug=False)

    x_d = nc.dram_tensor("x", [T, DIM], F32, kind="ExternalInput").ap()
    w_d = {}
    for name, shape in [("Wq", [DIM, DIM]), ("Wk", [DIM, DIM]),
                        ("Wv", [DIM, DIM]), ("Wo", [DIM, DIM]),
                        ("W1", [DIM, HID]), ("W2", [HID, DIM]),
                        ("bq", [DIM]), ("bk", [DIM]), ("bv", [DIM]),
                        ("bo", [DIM]), ("b1", [HID]), ("b2", [DIM]),
                        ("g1", [DIM]), ("be1", [DIM]), ("g2", [DIM]),
                        ("be2", [DIM])]:
        w_d[name] = nc.dram_tensor(name, shape, F32, kind="ExternalInput").ap()
    out_d = nc.dram_tensor("out", [T, DIM], F32, kind="ExternalOutput").ap()

    dbg = {}
    if DEBUG:
        for name, shape in [("dq", [DIM, PT]), ("dk", [DIM, PT]),
                            ("dctx", [DIM, PT]), ("dx1", [512, DIM]),
                            ("dh", [HID, PT])]:
            dbg[name] = nc.dram_tensor(name, shape, BF16, kind="ExternalOutput").ap()

    with tile.TileContext(nc) as tc, ExitStack() as octx:
        persist = octx.enter_context(tc.tile_pool(name="persist", bufs=1))
        stage = octx.enter_context(tc.tile_pool(name="stage", bufs=2))
        wpool = octx.enter_context(tc.tile_pool(name="weights", bufs=1))

        # ---------- constants ----------
        st0 = stage.tile([128, DIM], F32, tag="stg")
        make_identity(nc, st0[:, 0:128])
        identb = persist.tile([128, 128], BF16)
        nc.vector.tensor_copy(identb[:], st0[:, 0:128])
        eps_sb = persist.tile([128, 1], F32)
        nc.vector.memset(eps_sb[:], EPS)
        ones_row = persist.tile([1, HD], BF16)
        nc.vector.memset(ones_row[:], 1.0)

        # per-partition bias layouts [128, nchunk] fp32
        bq_sb = persist.tile([128, DC], F32)
        nc.sync.dma_start(bq_sb[:], w_d["bq"].rearrange("(c p) -> p c", p=128))
        bk_sb = persist.tile([128, DC], F32)
        nc.sync.dma_start(bk_sb[:], w_d["bk"].rearrange("(c p) -> p c", p=128))
        b1_sb = persist.tile([128, HC], F32)
        nc.sync.dma_start(b1_sb[:], w_d["b1"].rearrange("(c p) -> p c", p=128))
        b2_sb = persist.tile([128, DC], F32)
        nc.sync.dma_start(b2_sb[:], w_d["b2"].rearrange("(c p) -> p c", p=128))


        # ---------- weights: stage fp32 + cast to bf16 ----------
        wq = wpool.tile([128, DC, DIM], BF16, name="wq")
        wk = wpool.tile([128, DC, DIM], BF16, name="wk")
        wv = wpool.tile([128, DC, DIM], BF16, name="wv")
        wo = wpool.tile([128, DC, DIM], BF16, name="wo")
        w1b = wpool.tile([128, DC, HID], BF16, name="w1b")
        w2b = wpool.tile([128, HC, DIM], BF16, name="w2b")

        _cast_idx = [0]

        def _cast(dst_ap, src_ap):
            # alternate engines so neither queue serializes
            if _cast_idx[0] % 2 == 0:
                nc.vector.tensor_copy(dst_ap, src_ap)
            else:
                nc.scalar.activation(dst_ap, src_ap, AF.Copy, bias=0.0, scale=1.0)
            _cast_idx[0] += 1

        def _load_square(dst, src):
            # [768, 768] fp32 -> [128, 6, 768] bf16, one chunk per stage tile
            for j in range(DC):
                stw = stage.tile([128, DIM], F32, tag="stg")
                nc.sync.dma_start(stw[:], src[j * 128:(j + 1) * 128, :])
                _cast(dst[:, j, :], stw[:])


        def _load_w1():
            # [768, 3072] -> [128, 6, 3072] bf16, quarter-chunks of 768
            for c in range(DC):
                for h2 in range(4):
                    stw = stage.tile([128, DIM], F32, tag="stg")
                    nc.sync.dma_start(
                        stw[:],
                        w_d["W1"][c * 128:(c + 1) * 128,
                                  h2 * DIM:(h2 + 1) * DIM])
                    _cast(w1b[:, c, h2 * DIM:(h2 + 1) * DIM], stw[:])

        def _load_w2():
            # [3072, 768] -> [128, 24, 768] bf16, one chunk per stage tile
            for j in range(HC):
                stw = stage.tile([128, DIM], F32, tag="stg")
                nc.sync.dma_start(stw[:], w_d["W2"][j * 128:(j + 1) * 128, :])
                _cast(w2b[:, j, :], stw[:])

        # ---------- per-pair activation pools ----------
        xpool = octx.enter_context(tc.tile_pool(name="xsb", bufs=2))
        tmp32 = octx.enter_context(tc.tile_pool(name="tmp32", bufs=3))
        aopool = octx.enter_context(tc.tile_pool(name="aop", bufs=4))
        trpool = octx.enter_context(tc.tile_pool(name="trT", bufs=2))
        qkpool = octx.enter_context(tc.tile_pool(name="qk", bufs=1))
        vpool = octx.enter_context(tc.tile_pool(name="v", bufs=1))
        etpool = octx.enter_context(tc.tile_pool(name="et", bufs=3))
        cxpool = octx.enter_context(tc.tile_pool(name="cx", bufs=1))
        smpool = octx.enter_context(tc.tile_pool(name="sm", bufs=2))
        x1pool = octx.enter_context(tc.tile_pool(name="x1", bufs=1))
        htpool = octx.enter_context(tc.tile_pool(name="ht", bufs=3))
        mopool = octx.enter_context(tc.tile_pool(name="mo", bufs=2))
        mfull = octx.enter_context(tc.tile_pool(name="mfull", bufs=1))
        lnpool = octx.enter_context(tc.tile_pool(name="ln", bufs=2))
        mvpool = octx.enter_context(tc.tile_pool(name="mv4", bufs=4))

        ps_mm = octx.enter_context(tc.tile_pool(name="psmm", bufs=2, space="PSUM"))

        def _emit_x_load(p):
            """DMA pair p's x into f32 landing tiles, cast to bf16 x_sb."""
            g0 = p * PT
            x_sb = xpool.tile([128, 4, DIM], BF16, tag="x")
            for i, (off, sz) in enumerate(TOK_TILES):
                land = tmp32.tile([128, DIM], F32, tag="t32")
                nc.sync.dma_start(land[0:sz, :], x_d[g0 + off:g0 + off + sz, :])
                nc.vector.tensor_copy(x_sb[0:sz, i, :], land[0:sz, :])
            return x_sb

        x_next = _emit_x_load(0)
        _load_square(wq, w_d["Wq"])
        _load_square(wk, w_d["Wk"])

        # broadcast-[128, 768] bf16 tiles (staged fp32 -> cast)
        bcast = {}
        bias_names = ["bv", "bo", "g1", "be1", "g2", "be2"]
        for j, name in enumerate(bias_names):
            stb = stage.tile([128, DIM], F32, tag="stg")
            nc.sync.dma_start(stb[:],
                              w_d[name].unsqueeze(0).to_broadcast([128, DIM]))
            tb = persist.tile([128, DIM], BF16, name=f"bc_{name}")
            if j % 2 == 0:
                nc.vector.tensor_copy(tb[:], stb[:])
            else:
                nc.scalar.activation(tb[:], stb[:], AF.Copy, bias=0.0, scale=1.0)
            bcast[name] = tb
        _load_square(wv, w_d["Wv"])
        _load_square(wo, w_d["Wo"])

        def _emit_tail(tp, pacs, psac_ctx, x1t):
            """MLP2 eviction, transpose back, LN2 + residual, out DMA for
            pair tp. Emitted after pair tp+1's transpose/QKV phases so the
            PE never waits on this (DVE/Act-heavy) tail at pair boundaries."""
            tg0 = tp * PT
            mo = mfull.tile([128, 4, DIM], BF16, tag="mo")
            for c in range(DC):
                moTc = mopool.tile([128, PT], BF16, tag="moT")
                nc.scalar.activation(moTc[:, :], pacs[c][:, :], AF.Identity,
                                     bias=b2_sb[:, c:c + 1], scale=1.0)
                ptc = ps_mm.tile([128, 4, 128], BF16, tag="mm")
                for i, (off, sz) in enumerate(TOK_TILES):
                    nc.tensor.transpose(ptc[0:sz, i, :],
                                        moTc[:, off:off + sz],
                                        identb[:, :])
                nc.vector.tensor_copy(mo[:, 0:3, c * 128:(c + 1) * 128],
                                      ptc[:, 0:3, :])
                nc.vector.tensor_copy(mo[0:10, 3, c * 128:(c + 1) * 128],
                                      ptc[0:10, 3, :])
            psac_ctx.close()

            # LN2 + residual -> out (batched sqrt: avoid Exp/Sqrt table thrash)
            mvs2 = []
            vart = lnpool.tile([128, 4], F32, tag="vart")
            for i, (off, sz) in enumerate(TOK_TILES):
                st = lnpool.tile([128, 3, nc.vector.BN_STATS_DIM], F32, tag="st")
                for g in range(3):
                    nc.vector.bn_stats(st[0:sz, g, :],
                                       mo[0:sz, i, g * 256:(g + 1) * 256])
                mv = mvpool.tile([128, nc.vector.BN_AGGR_DIM], F32, tag="mv")
                nc.vector.bn_aggr(mv[0:sz, :], st[0:sz, :, :])
                nc.vector.tensor_copy(vart[0:sz, i:i + 1], mv[0:sz, 1:2])
                mvs2.append(mv)
            rstd = lnpool.tile([128, 4], F32, tag="rstd")
            nc.scalar.activation(rstd[:, :], vart[:, :], AF.Sqrt,
                                 bias=eps_sb[:, :], scale=1.0)
            nc.vector.reciprocal(rstd[:, :], rstd[:, :])
            for i, (off, sz) in enumerate(TOK_TILES):
                nmr = lnpool.tile([128, 1], F32, tag="nmr")
                nc.vector.tensor_scalar(nmr[0:sz, :], mvs2[i][0:sz, 0:1],
                                        rstd[0:sz, i:i + 1], -1.0, OP.mult, OP.mult)
                tln = tmp32.tile([128, DIM], F32, tag="t32")
                nc.scalar.activation(tln[0:sz, :], mo[0:sz, i, :], AF.Identity,
                                     bias=nmr[0:sz, :], scale=rstd[0:sz, i:i + 1])
                nc.vector.tensor_tensor(tln[0:sz, :], tln[0:sz, :],
                                        bcast["g2"][0:sz, :], OP.mult)
                xb = tmp32.tile([128, DIM], F32, tag="t32")
                nc.gpsimd.tensor_add(xb[0:sz, :], x1t[0:sz, i, :],
                                     bcast["be2"][0:sz, :])
                ot = tmp32.tile([128, DIM], F32, tag="t32")
                nc.vector.tensor_add(ot[0:sz, :], tln[0:sz, :], xb[0:sz, :])
                nc.sync.dma_start(out_d[tg0 + off:tg0 + off + sz, :], ot[0:sz, :])

        pending = None

        for p in range(NPAIR):
            g0 = p * PT
            x_sb = x_next

            # ---- prefetch next pair's x ----
            if p + 1 < NPAIR:
                x_next = _emit_x_load(p + 1)

            # ---- transpose x -> xT (d-major bf16) ----
            xT = trpool.tile([128, DC, PT], BF16, tag="trT")
            for i, (off, sz) in enumerate(TOK_TILES):
                for half in range(2):
                    pt = ps_mm.tile([128, 3, 128], BF16, tag="mm")
                    for cc in range(3):
                        c = half * 3 + cc
                        nc.tensor.transpose(pt[:, cc, 0:sz],
                                            x_sb[0:sz, i, c * 128:(c + 1) * 128],
                                            identb[0:sz, 0:sz])
                    nc.vector.tensor_copy(
                        xT[:, half * 3:half * 3 + 3, off:off + sz],
                        pt[:, :, 0:sz])

            # ---- Q/K projections (d-major bf16) ----
            qT = qkpool.tile([128, DC, PT], BF16, tag="qT")
            kT = qkpool.tile([128, DC, PT], BF16, tag="kT")
            for c in range(DC):
                for wt, bsb, dst in ((wq, bq_sb, qT), (wk, bk_sb, kT)):
                    pm = ps_mm.tile([128, PT], F32, tag="mm")
                    for kc in range(DC):
                        nc.tensor.matmul(pm[:, :],
                                         wt[:, kc, c * 128:(c + 1) * 128],
                                         xT[:, kc, :],
                                         start=(kc == 0), stop=(kc == DC - 1))
                    nc.scalar.activation(dst[:, c, :], pm[:, :], AF.Identity,
                                         bias=bsb[:, c:c + 1], scale=1.0)

            # ---- V projection (token-major, batch-aligned tiles) ----
            # 65th column holds ones so the ctx matmul also produces row-sums
            v_sb = vpool.tile([128, 4, NH, HD + 1], BF16, tag="v")
            nc.vector.memset(v_sb[:, :, :, HD:HD + 1], 1.0)
            for i, (off, sz) in enumerate(V_TILES):
                for s in range(2):
                    pm = ps_mm.tile([128, PT], F32, tag="mm")
                    for kc in range(DC):
                        nc.tensor.matmul(pm[0:sz, 0:384],
                                         xT[:, kc, off:off + sz],
                                         wv[:, kc, s * 384:(s + 1) * 384],
                                         start=(kc == 0), stop=(kc == DC - 1))
                    nc.vector.tensor_add(
                        v_sb[0:sz, i, 6 * s:6 * s + 6, 0:HD],
                        pm[0:sz, 0:384].rearrange("p (a b) -> p a b", a=6),
                        bcast["bv"][0:sz, s * 384:(s + 1) * 384]
                            .rearrange("p (a b) -> p a b", a=6))

            if p == 0:
                _load_w1()

            # ---- previous pair's MLP eviction + LN2 tail ----
            if pending is not None:
                _emit_tail(*pending)
                pending = None

            if DEBUG and p == 0:
                for c in range(DC):
                    nc.sync.dma_start(dbg["dq"][c * 128:(c + 1) * 128, :], qT[:, c, :])
                    nc.sync.dma_start(dbg["dk"][c * 128:(c + 1) * 128, :], kT[:, c, :])

            # ---- attention ----
            ctxT = cxpool.tile([128, DC, PT], BF16, tag="ctxT")
            with ExitStack() as actx:
                ps_at = actx.enter_context(
                    tc.tile_pool(name="psat", bufs=4, space="PSUM"))

                def _emit_scores(h):
                    """scores + exp for head h; returns (et_kt0, et_kt1)."""
                    hc, hp = h // 2, (h % 2) * 64
                    ets = []
                    for kt, (koff, ksz) in enumerate(KT_TILES):
                        psc = ps_at.tile([128, PT], F32, tag="at")
                        for b in range(2):
                            nc.tensor.matmul(
                                psc[0:ksz, b * S:(b + 1) * S],
                                kT[hp:hp + 64, hc, b * S + koff:b * S + koff + ksz],
                                qT[hp:hp + 64, hc, b * S:(b + 1) * S],
                                start=True, stop=True)
                        et = etpool.tile([128, PT], BF16, tag="et")
                        nc.scalar.activation(et[0:ksz, :], psc[0:ksz, :],
                                             AF.Exp, bias=0.0, scale=0.125)
                        ets.append(et)
                    return ets

                ets = _emit_scores(0)
                srf = None
                for h in range(NH):
                    hc, hp = h // 2, (h % 2) * 64
                    cur = ets
                    if h + 1 < NH:
                        ets = _emit_scores(h + 1)
                    # unnormalized ctx^T (row 64 = softmax denominators)
                    pc = ps_at.tile([128, PT], F32, tag="at")
                    for b in range(2):
                        for kt, (koff, ksz) in enumerate(KT_TILES):
                            nc.tensor.matmul(
                                pc[0:HD + 1, b * S:(b + 1) * S],
                                v_sb[0:ksz, 2 * b + kt, h, :],
                                cur[kt][0:ksz, b * S:(b + 1) * S],
                                start=(kt == 0), stop=(kt == 1))
                    nc.vector.tensor_copy(ctxT[hp:hp + 64, hc, :], pc[0:HD, :])
                    if hp == 0:
                        srf = smpool.tile([1, 2, PT], BF16, tag="srf")
                    nc.scalar.activation(srf[0:1, h % 2, :], pc[HD:HD + 1, :],
                                         AF.Copy, bias=0.0, scale=1.0)
                    if hp != 0:
                        # both heads of chunk hc done: broadcast their sums to
                        # 64 partitions each (contract-1 ones matmul), divide
                        pbc = ps_at.tile([128, PT], F32, tag="at")
                        for hh in range(2):
                            nc.tensor.matmul(pbc[hh * 64:hh * 64 + 64, :],
                                             ones_row[0:1, :],
                                             srf[0:1, hh, :],
                                             start=True, stop=True)
                        nc.vector.reciprocal_approx_fast(pbc[:, :], pbc[:, :])
                        nc.vector.tensor_tensor(ctxT[:, hc, :], ctxT[:, hc, :],
                                                pbc[:, :], OP.mult)

            if p == 0:
                _load_w2()

            if DEBUG and p == 0:
                for c in range(DC):
                    nc.sync.dma_start(dbg["dctx"][c * 128:(c + 1) * 128, :],
                                      ctxT[:, c, :])

            # ---- O-projection + LN1 + residual -> x1 (bf16) ----
            # pass 1: projections + stats; one batched sqrt for all 4 tiles
            # (Sqrt and Exp thrash the Act function table, so emit 1 not 4)
            x1 = x1pool.tile([128, 4, DIM], BF16, tag="x1")
            aos, mvs = [], []
            vart = lnpool.tile([128, 4], F32, tag="vart")
            for i, (off, sz) in enumerate(TOK_TILES):
                ao = aopool.tile([128, DIM], F32, tag="ao")
                for s in range(2):
                    pm = ps_mm.tile([128, PT], F32, tag="mm")
                    for c in range(DC):
                        nc.tensor.matmul(pm[0:sz, 0:384],
                                         ctxT[:, c, off:off + sz],
                                         wo[:, c, s * 384:(s + 1) * 384],
                                         start=(c == 0), stop=(c == DC - 1))
                    nc.vector.tensor_add(ao[0:sz, s * 384:(s + 1) * 384],
                                         pm[0:sz, 0:384],
                                         bcast["bo"][0:sz, s * 384:(s + 1) * 384])
                st = lnpool.tile([128, 3, nc.vector.BN_STATS_DIM], F32, tag="st")
                for g in range(3):
                    nc.vector.bn_stats(st[0:sz, g, :],
                                       ao[0:sz, g * 256:(g + 1) * 256])
                mv = mvpool.tile([128, nc.vector.BN_AGGR_DIM], F32, tag="mv")
                nc.vector.bn_aggr(mv[0:sz, :], st[0:sz, :, :])
                nc.vector.tensor_copy(vart[0:sz, i:i + 1], mv[0:sz, 1:2])
                aos.append(ao); mvs.append(mv)
            rstd = lnpool.tile([128, 4], F32, tag="rstd")
            nc.scalar.activation(rstd[:, :], vart[:, :], AF.Sqrt,
                                 bias=eps_sb[:, :], scale=1.0)
            nc.vector.reciprocal(rstd[:, :], rstd[:, :])
            for i, (off, sz) in enumerate(TOK_TILES):
                ao, mv = aos[i], mvs[i]
                nmr = lnpool.tile([128, 1], F32, tag="nmr")
                nc.vector.tensor_scalar(nmr[0:sz, :], mv[0:sz, 0:1],
                                        rstd[0:sz, i:i + 1], -1.0, OP.mult, OP.mult)
                tln = tmp32.tile([128, DIM], F32, tag="t32")
                nc.scalar.activation(tln[0:sz, :], ao[0:sz, :], AF.Identity,
                                     bias=nmr[0:sz, :], scale=rstd[0:sz, i:i + 1])
                nc.vector.tensor_tensor(tln[0:sz, :], tln[0:sz, :],
                                        bcast["g1"][0:sz, :], OP.mult)
                xb = tmp32.tile([128, DIM], F32, tag="t32")
                nc.gpsimd.tensor_add(xb[0:sz, :], x_sb[0:sz, i, :],
                                     bcast["be1"][0:sz, :])
                nc.vector.tensor_add(x1[0:sz, i, :], tln[0:sz, :], xb[0:sz, :])

            if DEBUG and p == 0:
                for i, (off, sz) in enumerate(TOK_TILES):
                    nc.sync.dma_start(dbg["dx1"][128 * i:128 * i + sz, :],
                                      x1[0:sz, i, :])

            # ---- transpose x1 -> x1T (d-major bf16) ----
            x1T = trpool.tile([128, DC, PT], BF16, tag="trT")
            for i, (off, sz) in enumerate(TOK_TILES):
                for half in range(2):
                    pt = ps_mm.tile([128, 3, 128], BF16, tag="mm")
                    for cc in range(3):
                        c = half * 3 + cc
                        nc.tensor.transpose(pt[:, cc, 0:sz],
                                            x1[0:sz, i, c * 128:(c + 1) * 128],
                                            identb[0:sz, 0:sz])
                    nc.vector.tensor_copy(
                        x1T[:, half * 3:half * 3 + 3, off:off + sz],
                        pt[:, :, 0:sz])

            # ---- MLP: interleaved MLP1 (gelu) and MLP2 (6-bank acc) ----
            psac_ctx = ExitStack()
            ps_ac = psac_ctx.enter_context(
                tc.tile_pool(name=f"psac{p}", bufs=6, space="PSUM"))
            pacs = [ps_ac.tile([128, PT], F32, tag="ac", name=f"pac{c}")
                    for c in range(DC)]

            def _emit_mlp1(hcx):
                pm = ps_mm.tile([128, PT], F32, tag="mm")
                for kc in range(DC):
                    nc.tensor.matmul(pm[:, :],
                                     w1b[:, kc, hcx * 128:(hcx + 1) * 128],
                                     x1T[:, kc, :],
                                     start=(kc == 0), stop=(kc == DC - 1))
                hTc = htpool.tile([128, PT], BF16, tag="hT")
                nc.scalar.activation(hTc[:, :], pm[:, :], AF.Gelu,
                                     bias=b1_sb[:, hcx:hcx + 1], scale=1.0)
                return hTc

            hT_cur = _emit_mlp1(0)
            for hcx in range(HC):
                hT_use = hT_cur
                if hcx + 1 < HC:
                    hT_cur = _emit_mlp1(hcx + 1)
                for c in range(DC):
                    nc.tensor.matmul(pacs[c][:, :],
                                     w2b[:, hcx, c * 128:(c + 1) * 128],
                                     hT_use[:, :],
                                     start=(hcx == 0), stop=(hcx == HC - 1))
                if DEBUG and p == 0:
                    nc.sync.dma_start(
                        dbg["dh"][hcx * 128:(hcx + 1) * 128, :], hT_use[:, :])

            pending = (p, pacs, psac_ctx, x1)

        # tail of the last pair
        _emit_tail(*pending)

    nc.compile()
    return nc


def kernel(x, Wq, bq, Wk, bk, Wv, bv, Wo, bo, W1, b1, W2, b2, g1, be1, g2, be2):
    global _cached
    if _cached is None:
        _cached = _build()
    nc = _cached

    weights = dict(Wq=Wq, bq=bq, Wk=Wk, bk=bk, Wv=Wv, bv=bv, Wo=Wo, bo=bo,
                   W1=W1, b1=b1, W2=W2, b2=b2, g1=g1, be1=be1, g2=g2, be2=be2)
    weights = {k: np.ascontiguousarray(v, dtype=np.float32) for k, v in weights.items()}
    x = np.asarray(x, dtype=np.float32)

    in_maps = []
    for c in range(N_CORES):
        xc = np.ascontiguousarray(
            x[c * BPC:(c + 1) * BPC].reshape(T, DIM))
        in_maps.append({"x": xc, **weights})

    res = run_bass_kernel_spmd(nc, in_maps, core_ids=list(range(N_CORES)),
                               trace=bool(int(os.environ.get("BASSK_TRACE", "0"))))
    kernel._last_res = res
    out = np.concatenate(
        [res.results[c]["out"].reshape(BPC, S, DIM) for c in range(N_CORES)], axis=0)
    return out.astype(np.float32)


# revision 26
# speedup vs baseline: 1.6557x; 1.0117x over previous
"""Trainium2 Bass kernel for a ViT-Base transformer encoder block.

Input x: [64, 197, 768] fp32 + weights. Data-parallel over batch across 8
NeuronCores (8 batches/core = 1576 tokens/core). Single fused pass per
batch-pair (394 tokens), 4 pairs per core:

  x -> (bf16) xT -> Q/K/V projections (bf16 matmuls) -> attention with
  per-batch moving dim 197 (bf16), row-sums accumulated into a shared PSUM
  via ones-matmuls, one reciprocal for all heads, PE-broadcast of the
  per-(head,token) reciprocals via a 0/1 selection matrix -> O-projection ->
  LN1+residual -> x1 (bf16, kept in SBUF) -> MLP (bf16, W1/W2 resident,
  interleaved MLP1/MLP2 with 6-bank PSUM accumulation) -> transpose back ->
  LN2+residual -> out.

All weights are converted to bf16 on-chip once (staged fp32 DMA + cast).
"""
import os
import sys

sys.path.insert(0, "/opt/trn_rl_repo")

import numpy as np
from contextlib import ExitStack

import concourse.bass as bass
import concourse.tile as tile
from concourse import bacc, mybir
from concourse.bass_utils import run_bass_kernel_spmd
from concourse.masks import make_identity

DIM, NH, HD, HID = 768, 12, 64, 3072
S = 197
B = 64
N_CORES = 8
BPC = B // N_CORES            # 8 batches per core
T = BPC * S                   # 1576 tokens per core
NPAIR = BPC // 2              # 4 batch pairs per core
PT = 2 * S                    # 394 tokens per pair
EPS = 1e-6
DC = DIM // 128               # 6 d-chunks
HC = HID // 128               # 24 hidden chunks

F32 = mybir.dt.float32
F32R = mybir.dt.float32r
BF16 = mybir.dt.bfloat16
AF = mybir.ActivationFunctionType
OP = mybir.AluOpType

# 128-aligned token tiles within a pair (for x/LN/O/out)
TOK_TILES = [(0, 128), (128, 128), (256, 128), (384, 10)]
# batch-aligned token tiles (for K/V in attention); tile 2*b + kt
V_TILES = [(0, 128), (128, 69), (197, 128), (325, 69)]
KT_TILES = [(0, 128), (128, 69)]  # (offset within batch, size)

DEBUG = bool(int(os.environ.get("BASSK_DEBUG", "0")))

_cached = None


def _build():
    nc = bacc.Bacc("TRN2", target_bir_lowering=False, debug=False)

    x_d = nc.dram_tensor("x", [T, DIM], F32, kind="ExternalInput").ap()
    w_d = {}
    for name, shape in [("Wq", [DIM, DIM]), ("Wk", [DIM, DIM]),
                        ("Wv", [DIM, DIM]), ("Wo", [DIM, DIM]),
                        ("W1", [DIM, HID]), ("W2", [HID, DIM]),
                        ("bq", [DIM]), ("bk", [DIM]), ("bv", [DIM]),
                        ("bo", [DIM]), ("b1", [HID]), ("b2", [DIM]),
                        ("g1", [DIM]), ("be1", [DIM]), ("g2", [DIM]),
                        ("be2", [DIM])]:
        w_d[name] = nc.dram_tensor(name, shape, F32, kind="ExternalInput").ap()
    out_d = nc.dram_tensor("out", [T, DIM], F32, kind="ExternalOutput").ap()

    dbg = {}
    if DEBUG:
        for name, shape in [("dq", [DIM, PT]), ("dk", [DIM, PT]),
                            ("dctx", [DIM, PT]), ("dx1", [512, DIM]),
                            ("dh", [HID, PT])]:
            dbg[name] = nc.dram_tensor(name, shape, BF16, kind="ExternalOutput").ap()

    with tile.TileContext(nc) as tc, ExitStack() as octx:
        persist = octx.enter_context(tc.tile_pool(name="persist", bufs=1))
        stage = octx.enter_context(tc.tile_pool(name="stage", bufs=2))
        wpool = octx.enter_context(tc.tile_pool(name="weights", bufs=1))

        # ---------- constants ----------
        st0 = stage.tile([128, DIM], F32, tag="stg")
        make_identity(nc, st0[:, 0:128])
        identb = persist.tile([128, 128], BF16)
        nc.vector.tensor_copy(identb[:], st0[:, 0:128])
        eps_sb = persist.tile([128, 1], F32)
        nc.vector.memset(eps_sb[:], EPS)
        ones_row = persist.tile([1, HD], BF16)
        nc.vector.memset(ones_row[:], 1.0)

        # per-partition bias layouts [128, nchunk] fp32
        bq_sb = persist.tile([128, DC], F32)
        nc.sync.dma_start(bq_sb[:], w_d["bq"].rearrange("(c p) -> p c", p=128))
        bk_sb = persist.tile([128, DC], F32)
        nc.sync.dma_start(bk_sb[:], w_d["bk"].rearrange("(c p) -> p c", p=128))
        b1_sb = persist.tile([128, HC], F32)
        nc.sync.dma_start(b1_sb[:], w_d["b1"].rearrange("(c p) -> p c", p=128))
        b2_sb = persist.tile([128, DC], F32)
        nc.sync.dma_start(b2_sb[:], w_d["b2"].rearrange("(c p) -> p c", p=128))


        # ---------- weights: stage fp32 + cast to bf16 ----------
        wq = wpool.tile([128, DC, DIM], BF16, name="wq")
        wk = wpool.tile([128, DC, DIM], BF16, name="wk")
        wv = wpool.tile([128, DC, DIM], BF16, name="wv")
        wo = wpool.tile([128, DC, DIM], BF16, name="wo")
        w1b = wpool.tile([128, DC, HID], BF16, name="w1b")
        w2b = wpool.tile([128, HC, DIM], BF16, name="w2b")

        _cast_idx = [0]

        def _cast(dst_ap, src_ap):
            # alternate engines so neither queue serializes
            if _cast_idx[0] % 2 == 0:
                nc.vector.tensor_copy(dst_ap, src_ap)
            else:
                nc.scalar.activation(dst_ap, src_ap, AF.Copy, bias=0.0, scale=1.0)
            _cast_idx[0] += 1

        def _load_square(dst, src):
            # [768, 768] fp32 -> [128, 6, 768] bf16, one chunk per stage tile
            for j in range(DC):
                stw = stage.tile([128, DIM], F32, tag="stg")
                nc.sync.dma_start(stw[:], src[j * 128:(j + 1) * 128, :])
                _cast(dst[:, j, :], stw[:])


        def _load_w1():
            # [768, 3072] -> [128, 6, 3072] bf16, quarter-chunks of 768
            for c in range(DC):
                for h2 in range(4):
                    stw = stage.tile([128, DIM], F32, tag="stg")
                    nc.sync.dma_start(
                        stw[:],
                        w_d["W1"][c * 128:(c + 1) * 128,
                                  h2 * DIM:(h2 + 1) * DIM])
                    _cast(w1b[:, c, h2 * DIM:(h2 + 1) * DIM], stw[:])

        def _load_w2():
            # [3072, 768] -> [128, 24, 768] bf16, one chunk per stage tile
            for j in range(HC):
                stw = stage.tile([128, DIM], F32, tag="stg")
                nc.sync.dma_start(stw[:], w_d["W2"][j * 128:(j + 1) * 128, :])
                _cast(w2b[:, j, :], stw[:])

        # ---------- per-pair activation pools ----------
        xpool = octx.enter_context(tc.tile_pool(name="xsb", bufs=2))
        tmp32 = octx.enter_context(tc.tile_pool(name="tmp32", bufs=3))
        aopool = octx.enter_context(tc.tile_pool(name="aop", bufs=4))
        trpool = octx.enter_context(tc.tile_pool(name="trT", bufs=2))
        qkpool = octx.enter_context(tc.tile_pool(name="qk", bufs=1))
        vpool = octx.enter_context(tc.tile_pool(name="v", bufs=1))
        etpool = octx.enter_context(tc.tile_pool(name="et", bufs=3))
        cxpool = octx.enter_context(tc.tile_pool(name="cx", bufs=1))
        smpool = octx.enter_context(tc.tile_pool(name="sm", bufs=2))
        x1pool = octx.enter_context(tc.tile_pool(name="x1", bufs=1))
        htpool = octx.enter_context(tc.tile_pool(name="ht", bufs=3))
        mopool = octx.enter_context(tc.tile_pool(name="mo", bufs=2))
        mfull = octx.enter_context(tc.tile_pool(name="mfull", bufs=1))
        lnpool = octx.enter_context(tc.tile_pool(name="ln", bufs=2))
        mvpool = octx.enter_context(tc.tile_pool(name="mv4", bufs=4))

        ps_mm = octx.enter_context(tc.tile_pool(name="psmm", bufs=2, space="PSUM"))

        def _emit_x_load(p):
            """DMA pair p's x into f32 landing tiles, cast to bf16 x_sb."""
            g0 = p * PT
            x_sb = xpool.tile([128, 4, DIM], BF16, tag="x")
            for i, (off, sz) in enumerate(TOK_TILES):
                land = tmp32.tile([128, DIM], F32, tag="t32")
                nc.sync.dma_start(land[0:sz, :], x_d[g0 + off:g0 + off + sz, :])
                nc.vector.tensor_copy(x_sb[0:sz, i, :], land[0:sz, :])
            return x_sb

        x_next = _emit_x_load(0)
        _load_square(wq, w_d["Wq"])
        _load_square(wk, w_d["Wk"])

        # broadcast-[128, 768] bf16 tiles (staged fp32 -> cast)
        bcast = {}
        bias_names = ["bv", "bo", "g1", "be1", "g2", "be2"]
        for j, name in enumerate(bias_names):
            stb = stage.tile([128, DIM], F32, tag="stg")
            nc.sync.dma_start(stb[:],
                              w_d[name].unsqueeze(0).to_broadcast([128, DIM]))
            tb = persist.tile([128, DIM], BF16, name=f"bc_{name}")
            if j % 2 == 0:
                nc.vector.tensor_copy(tb[:], stb[:])
            else:
                nc.scalar.activation(tb[:], stb[:], AF.Copy, bias=0.0, scale=1.0)
            bcast[name] = tb
        _load_square(wv, w_d["Wv"])
        _load_square(wo, w_d["Wo"])

        def _emit_tail(tp, pacs, psac_ctx, x1t):
            """MLP2 eviction, transpose back, LN2 + residual, out DMA for
            pair tp. Emitted after pair tp+1's transpose/QKV phases so the
            PE never waits on this (DVE/Act-heavy) tail at pair boundaries."""
            tg0 = tp * PT
            mo = mfull.tile([128, 4, DIM], BF16, tag="mo")
            for c in range(DC):
                moTc = mopool.tile([128, PT], BF16, tag="moT")
                nc.scalar.activation(moTc[:, :], pacs[c][:, :], AF.Identity,
                                     bias=b2_sb[:, c:c + 1], scale=1.0)
                ptc = ps_mm.tile([128, 4, 128], BF16, tag="mm")
                for i, (off, sz) in enumerate(TOK_TILES):
                    nc.tensor.transpose(ptc[0:sz, i, :],
                                        moTc[:, off:off + sz],
                                        identb[:, :])
                nc.vector.tensor_copy(mo[:, 0:3, c * 128:(c + 1) * 128],
                                      ptc[:, 0:3, :])
                nc.vector.tensor_copy(mo[0:10, 3, c * 128:(c + 1) * 128],
                                      ptc[0:10, 3, :])
            psac_ctx.close()

            # LN2 + residual -> out (batched sqrt: avoid Exp/Sqrt table thrash)
            mvs2 = []
            vart = lnpool.tile([128, 4], F32, tag="vart")
            for i, (off, sz) in enumerate(TOK_TILES):
                st = lnpool.tile([128, 3, nc.vector.BN_STATS_DIM], F32, tag="st")
                for g in range(3):
                    nc.vector.bn_stats(st[0:sz, g, :],
                                       mo[0:sz, i, g * 256:(g + 1) * 256])
                mv = mvpool.tile([128, nc.vector.BN_AGGR_DIM], F32, tag="mv")
                nc.vector.bn_aggr(mv[0:sz, :], st[0:sz, :, :])
                nc.vector.tensor_copy(vart[0:sz, i:i + 1], mv[0:sz, 1:2])
                mvs2.append(mv)
            rstd = lnpool.tile([128, 4], F32, tag="rstd")
            nc.scalar.activation(rstd[:, :], vart[:, :], AF.Sqrt,
                                 bias=eps_sb[:, :], scale=1.0)
            nc.vector.reciprocal(rstd[:, :], rstd[:, :])
            for i, (off, sz) in enumerate(TOK_TILES):
                nmr = lnpool.tile([128, 1], F32, tag="nmr")
                nc.vector.tensor_scalar(nmr[0:sz, :], mvs2[i][0:sz, 0:1],
                                        rstd[0:sz, i:i + 1], -1.0, OP.mult, OP.mult)
                tln = tmp32.tile([128, DIM], F32, tag="t32")
                nc.scalar.activation(tln[0:sz, :], mo[0:sz, i, :], AF.Identity,
                                     bias=nmr[0:sz, :], scale=rstd[0:sz, i:i + 1])
                nc.vector.tensor_tensor(tln[0:sz, :], tln[0:sz, :],
                                        bcast["g2"][0:sz, :], OP.mult)
                xb = tmp32.tile([128, DIM], F32, tag="t32")
                nc.gpsimd.tensor_add(xb[0:sz, :], x1t[0:sz, i, :],
                                     bcast["be2"][0:sz, :])
                ot = tmp32.tile([128, DIM], F32, tag="t32")
                nc.vector.tensor_add(ot[0:sz, :], tln[0:sz, :], xb[0:sz, :])
                nc.sync.dma_start(out_d[tg0 + off:tg0 + off + sz, :], ot[0:sz, :])

        pending = None

        for p in range(NPAIR):
            g0 = p * PT
            x_sb = x_next

            # ---- prefetch next pair's x ----
            if p + 1 < NPAIR:
                x_next = _emit_x_load(p + 1)

            # ---- transpose x -> xT (d-major bf16) ----
            xT = trpool.tile([128, DC, PT], BF16, tag="trT")
            for i, (off, sz) in enumerate(TOK_TILES):
                for half in range(2):
                    pt = ps_mm.tile([128, 3, 128], BF16, tag="mm")
                    for cc in range(3):
                        c = half * 3 + cc
                        nc.tensor.transpose(pt[:, cc, 0:sz],
                                            x_sb[0:sz, i, c * 128:(c + 1) * 128],
                                            identb[0:sz, 0:sz])
                    nc.vector.tensor_copy(
                        xT[:, half * 3:half * 3 + 3, off:off + sz],
                        pt[:, :, 0:sz])

            # ---- Q/K projections (d-major bf16) ----
            qT = qkpool.tile([128, DC, PT], BF16, tag="qT")
            kT = qkpool.tile([128, DC, PT], BF16, tag="kT")
            for c in range(DC):
                for wt, bsb, dst in ((wq, bq_sb, qT), (wk, bk_sb, kT)):
                    pm = ps_mm.tile([128, PT], F32, tag="mm")
                    for kc in range(DC):
                        nc.tensor.matmul(pm[:, :],
                                         wt[:, kc, c * 128:(c + 1) * 128],
                                         xT[:, kc, :],
                                         start=(kc == 0), stop=(kc == DC - 1))
                    nc.scalar.activation(dst[:, c, :], pm[:, :], AF.Identity,
                                         bias=bsb[:, c:c + 1], scale=1.0)

            # ---- V projection (token-major, batch-aligned tiles) ----
            # 65th column holds ones so the ctx matmul also produces row-sums
            v_sb = vpool.tile([128, 4, NH, HD + 1], BF16, tag="v")
            nc.vector.memset(v_sb[:, :, :, HD:HD + 1], 1.0)
            for i, (off, sz) in enumerate(V_TILES):
                for s in range(2):
                    pm = ps_mm.tile([128, PT], F32, tag="mm")
                    for kc in range(DC):
                        nc.tensor.matmul(pm[0:sz, 0:384],
                                         xT[:, kc, off:off + sz],
                                         wv[:, kc, s * 384:(s + 1) * 384],
                                         start=(kc == 0), stop=(kc == DC - 1))
                    nc.vector.tensor_add(
                        v_sb[0:sz, i, 6 * s:6 * s + 6, 0:HD],
                        pm[0:sz, 0:384].rearrange("p (a b) -> p a b", a=6),
                        bcast["bv"][0:sz, s * 384:(s + 1) * 384]
                            .rearrange("p (a b) -> p a b", a=6))

            # ---- previous pair's MLP eviction + LN2 tail ----
            if pending is not None:
                _emit_tail(*pending)
                pending = None

            if DEBUG and p == 0:
                for c in range(DC):
                    nc.sync.dma_start(dbg["dq"][c * 128:(c + 1) * 128, :], qT[:, c, :])
                    nc.sync.dma_start(dbg["dk"][c * 128:(c + 1) * 128, :], kT[:, c, :])

            # ---- attention ----
            ctxT = cxpool.tile([128, DC, PT], BF16, tag="ctxT")
            with ExitStack() as actx:
                ps_at = actx.enter_context(
                    tc.tile_pool(name="psat", bufs=4, space="PSUM"))

                def _emit_scores(h):
                    """scores + exp for head h; returns (et_kt0, et_kt1)."""
                    hc, hp = h // 2, (h % 2) * 64
                    ets = []
                    for kt, (koff, ksz) in enumerate(KT_TILES):
                        psc = ps_at.tile([128, PT], F32, tag="at")
                        for b in range(2):
                            nc.tensor.matmul(
                                psc[0:ksz, b * S:(b + 1) * S],
                                kT[hp:hp + 64, hc, b * S + koff:b * S + koff + ksz],
                                qT[hp:hp + 64, hc, b * S:(b + 1) * S],
                                start=True, stop=True)
                        et = etpool.tile([128, PT], BF16, tag="et")
                        nc.scalar.activation(et[0:ksz, :], psc[0:ksz, :],
                                             AF.Exp, bias=0.0, scale=0.125)
                        ets.append(et)
                    return ets

                ets = _emit_scores(0)
                srf = None
                for h in range(NH):
                    hc, hp = h // 2, (h % 2) * 64
                    cur = ets
                    if h + 1 < NH:
                        ets = _emit_scores(h + 1)
                    # unnormalized ctx^T (row 64 = softmax denominators)
                    pc = ps_at.tile([128, PT], F32, tag="at")
                    for b in range(2):
                        for kt, (koff, ksz) in enumerate(KT_TILES):
                            nc.tensor.matmul(
                                pc[0:HD + 1, b * S:(b + 1) * S],
                                v_sb[0:ksz, 2 * b + kt, h, :],
                                cur[kt][0:ksz, b * S:(b + 1) * S],
                                start=(kt == 0), stop=(kt == 1))
                    nc.vector.tensor_copy(ctxT[hp:hp + 64, hc, :], pc[0:HD, :])
                    if hp == 0:
                        srf = smpool.tile([1, 2, PT], BF16, tag="srf")
                    nc.scalar.activation(srf[0:1, h % 2, :], pc[HD:HD + 1, :],
                                         AF.Copy, bias=0.0, scale=1.0)
                    if hp != 0:
                        # both heads of chunk hc done: broadcast their sums to
                        # 64 partitions each (contract-1 ones matmul), divide
                        pbc = ps_at.tile([128, PT], F32, tag="at")
                        for hh in range(2):
                            nc.tensor.matmul(pbc[hh * 64:hh * 64 + 64, :],
                                             ones_row[0:1, :],
                                             srf[0:1, hh, :],
                                             start=True, stop=True)
                        nc.vector.reciprocal_approx_fast(pbc[:, :], pbc[:, :])
                        nc.vector.tensor_tensor(ctxT[:, hc, :], ctxT[:, hc, :],
                                                pbc[:, :], OP.mult)

            if p == 0:
                _load_w1()

            if DEBUG and p == 0:
                for c in range(DC):
                    nc.sync.dma_start(dbg["dctx"][c * 128:(c + 1) * 128, :],
                                      ctxT[:, c, :])

            # ---- O-projection + LN1 + residual -> x1 (bf16) ----
            # pass 1: projections + stats; one batched sqrt for all 4 tiles
            # (Sqrt and Exp thrash the Act function table, so emit 1 not 4)
            x1 = x1pool.tile([128, 4, DIM], BF16, tag="x1")
            for i, (off, sz) in enumerate(TOK_TILES):
                ao = aopool.tile([128, DIM], F32, tag="ao")
                for s in range(2):
                    pm = ps_mm.tile([128, PT], F32, tag="mm")
                    for c in range(DC):
                        nc.tensor.matmul(pm[0:sz, 0:384],
                                         ctxT[:, c, off:off + sz],
                                         wo[:, c, s * 384:(s + 1) * 384],
                                         start=(c == 0), stop=(c == DC - 1))
                    nc.vector.tensor_add(ao[0:sz, s * 384:(s + 1) * 384],
                                         pm[0:sz, 0:384],
                                         bcast["bo"][0:sz, s * 384:(s + 1) * 384])
                st = lnpool.tile([128, 3, nc.vector.BN_STATS_DIM], F32, tag="st")
                for g in range(3):
                    nc.vector.bn_stats(st[0:sz, g, :],
                                       ao[0:sz, g * 256:(g + 1) * 256])
                mv = mvpool.tile([128, nc.vector.BN_AGGR_DIM], F32, tag="mv")
                nc.vector.bn_aggr(mv[0:sz, :], st[0:sz, :, :])
                rstd = lnpool.tile([128, 1], F32, tag="rstd")
                nc.scalar.activation(rstd[0:sz, :], mv[0:sz, 1:2], AF.Sqrt,
                                     bias=eps_sb[0:sz, :], scale=1.0)
                nc.vector.reciprocal(rstd[0:sz, :], rstd[0:sz, :])
                nmr = lnpool.tile([128, 1], F32, tag="nmr")
                nc.vector.tensor_scalar(nmr[0:sz, :], mv[0:sz, 0:1],
                                        rstd[0:sz, :], -1.0, OP.mult, OP.mult)
                tln = tmp32.tile([128, DIM], F32, tag="t32")
                nc.scalar.activation(tln[0:sz, :], ao[0:sz, :], AF.Identity,
                                     bias=nmr[0:sz, :], scale=rstd[0:sz, :])
                nc.vector.tensor_tensor(tln[0:sz, :], tln[0:sz, :],
                                        bcast["g1"][0:sz, :], OP.mult)
                xb = tmp32.tile([128, DIM], F32, tag="t32")
                nc.gpsimd.tensor_add(xb[0:sz, :], x_sb[0:sz, i, :],
                                     bcast["be1"][0:sz, :])
                nc.vector.tensor_add(x1[0:sz, i, :], tln[0:sz, :], xb[0:sz, :])

            if DEBUG and p == 0:
                for i, (off, sz) in enumerate(TOK_TILES):
                    nc.sync.dma_start(dbg["dx1"][128 * i:128 * i + sz, :],
                                      x1[0:sz, i, :])

            # ---- transpose x1 -> x1T (d-major bf16) ----
            x1T = trpool.tile([128, DC, PT], BF16, tag="trT")
            for i, (off, sz) in enumerate(TOK_TILES):
                for half in range(2):
                    pt = ps_mm.tile([128, 3, 128], BF16, tag="mm")
                    for cc in range(3):
                        c = half * 3 + cc
                        nc.tensor.transpose(pt[:, cc, 0:sz],
                                            x1[0:sz, i, c * 128:(c + 1) * 128],
                                            identb[0:sz, 0:sz])
                    nc.vector.tensor_copy(
                        x1T[:, half * 3:half * 3 + 3, off:off + sz],
                        pt[:, :, 0:sz])

            if p == 0:
                _load_w2()

            # ---- MLP: interleaved MLP1 (gelu) and MLP2 (6-bank acc) ----
            psac_ctx = ExitStack()
            ps_ac = psac_ctx.enter_context(
                tc.tile_pool(name=f"psac{p}", bufs=6, space="PSUM"))
            pacs = [ps_ac.tile([128, PT], F32, tag="ac", name=f"pac{c}")
                    for c in range(DC)]

            def _emit_mlp1(hcx):
                pm = ps_mm.tile([128, PT], F32, tag="mm")
                for kc in range(DC):
                    nc.tensor.matmul(pm[:, :],
                                     w1b[:, kc, hcx * 128:(hcx + 1) * 128],
                                     x1T[:, kc, :],
                                     start=(kc == 0), stop=(kc == DC - 1))
                hTc = htpool.tile([128, PT], BF16, tag="hT")
                nc.scalar.activation(hTc[:, :], pm[:, :], AF.Gelu,
                                     bias=b1_sb[:, hcx:hcx + 1], scale=1.0)
                return hTc

            hT_cur = _emit_mlp1(0)
            for hcx in range(HC):
                hT_use = hT_cur
                if hcx + 1 < HC:
                    hT_cur = _emit_mlp1(hcx + 1)
                for c in range(DC):
                    nc.tensor.matmul(pacs[c][:, :],
                                     w2b[:, hcx, c * 128:(c + 1) * 128],
                                     hT_use[:, :],
                                     start=(hcx == 0), stop=(hcx == HC - 1))
                if DEBUG and p == 0:
                    nc.sync.dma_start(
                        dbg["dh"][hcx * 128:(hcx + 1) * 128, :], hT_use[:, :])

            pending = (p, pacs, psac_ctx, x1)

        # tail of the last pair
        _emit_tail(*pending)

    nc.compile()
    return nc


def kernel(x, Wq, bq, Wk, bk, Wv, bv, Wo, bo, W1, b1, W2, b2, g1, be1, g2, be2):
    global _cached
    if _cached is None:
        _cached = _build()
    nc = _cached

    weights = dict(Wq=Wq, bq=bq, Wk=Wk, bk=bk, Wv=Wv, bv=bv, Wo=Wo, bo=bo,
                   W1=W1, b1=b1, W2=W2, b2=b2, g1=g1, be1=be1, g2=g2, be2=be2)
    weights = {k: np.ascontiguousarray(v, dtype=np.float32) for k, v in weights.items()}
    x = np.asarray(x, dtype=np.float32)

    in_maps = []
    for c in range(N_CORES):
        xc = np.ascontiguousarray(
            x[c * BPC:(c + 1) * BPC].reshape(T, DIM))
        in_maps.append({"x": xc, **weights})

    res = run_bass_kernel_spmd(nc, in_maps, core_ids=list(range(N_CORES)),
                               trace=bool(int(os.environ.get("BASSK_TRACE", "0"))))
    kernel._last_res = res
    out = np.concatenate(
        [res.results[c]["out"].reshape(BPC, S, DIM) for c in range(N_CORES)], axis=0)
    return out.astype(np.float32)
